# revision 17
# baseline (speedup 1.0000x reference)
"""MLA attention kernel for Trainium2 — 8-core tensor-parallel (self-contained).

Sharding: data-parallel over batch (2) x tensor-parallel over head groups
(4 groups of 4 heads) = 8 cores, SPMD (one NEFF, per-core input shards).
Core ci: batch ci//4, heads [4*(ci%4), 4*(ci%4)+4).

Per-core dataflow (everything feature-major "transposed" so the PE never
needs an on-chip transpose):
  x^T tiles via bf16 DMA-transpose (prefetched one chunk ahead)
  kv^T = wkvd.T @ x^T (rmsnorm sum via ones-matmul, scale broadcast via K=1 matmul)
  q_lat^T = wqd.T @ x^T ; q^T = wqu.T @ q_lat^T ; k_nope^T = wkvuk.T @ kv_c^T
  v (token-major) = kv_c^T.T @ wkvuv
  RoPE on rope rows (DVE); host-permuted weight columns group x1/x2 rows
  scores^T[k,q] = k^T.T @ q^T -> exp (ACT, scale folded) -> causal mask (DVE)
  denom[1,q] = ones.T @ E ; out_h^T[v,q] = v.T @ E   (both pipelined 3 deep)
  normalize via K=1 broadcast matmul of 1/denom, deferred one iteration
  out[t, hid] = attn^T.T @ w_out (token-major, contiguous writes)
Host: sums the 4 partial outputs per batch.
"""

import math

import numpy as np
import ml_dtypes

# ---- problem constants (from the reference model) ----
B, S, HID = 2, 2048, 2048
H, D_NOPE, D_ROPE, V_DIM = 16, 128, 64, 128
KV_RANK, Q_RANK = 512, 1536
HEAD_DIM = D_NOPE + D_ROPE
THETA, EPS = 10000.0, 1e-6
NCORES = 8
NH = 4                    # heads per core
T = 512                   # phase-A token chunk
NT = S // T
QC = 512                  # attention query chunk
NQC = S // QC
KH = HID // 128           # 16 k-chunks over HID
RQ = Q_RANK // 128        # 12 chunks over q rank
RKV = KV_RANK // 128      # 4 chunks over kv rank
SCALE = 1.0 / math.sqrt(HEAD_DIM)

_CACHE = {}


def build_nc(taps=False):
    """Build the Bass/Tile program (one NeuronCore, run SPMD on 8)."""
    from contextlib import ExitStack

    import concourse.mybir as mybir
    import concourse.tile as tile
    from concourse import bacc
    from concourse.bass import ds

    dt = mybir.dt
    AF = mybir.ActivationFunctionType
    bf16 = dt.bfloat16
    f32 = dt.float32

    nc = bacc.Bacc(
        "TRN2",
        target_bir_lowering=False,
        debug=False,
        enable_asserts=False,
        num_devices=NCORES,
    )

    # ---- I/O ----
    x_ap = nc.dram_tensor("x", [S, HID], bf16, kind="ExternalInput").ap()
    wqd_ap = nc.dram_tensor("wqd", [HID, Q_RANK], bf16, kind="ExternalInput").ap()
    wqu_ap = nc.dram_tensor("wqu", [Q_RANK, NH * HEAD_DIM], bf16, kind="ExternalInput").ap()
    wkvd_ap = nc.dram_tensor("wkvd", [HID, KV_RANK + D_ROPE], bf16, kind="ExternalInput").ap()
    wkvuk_ap = nc.dram_tensor("wkvuk", [KV_RANK, NH * D_NOPE], bf16, kind="ExternalInput").ap()
    wkvuv_ap = nc.dram_tensor("wkvuv", [KV_RANK, NH * V_DIM], bf16, kind="ExternalInput").ap()
    wout_ap = nc.dram_tensor("wout", [NH * V_DIM, HID], bf16, kind="ExternalInput").ap()
    cos_ap = nc.dram_tensor("cosq", [128, S], bf16, kind="ExternalInput").ap()
    sin_ap = nc.dram_tensor("sinq", [128, S], bf16, kind="ExternalInput").ap()
    mask_ap = nc.dram_tensor("maskt", [128, 1024], bf16, kind="ExternalInput").ap()
    onesc_ap = nc.dram_tensor("ones_col", [128, 1], bf16, kind="ExternalInput").ap()
    onesr_ap = nc.dram_tensor("ones_row", [1, 128], f32, kind="ExternalInput").ap()
    out_ap = nc.dram_tensor("out", [S, HID], f32, kind="ExternalOutput").ap()

    with tile.TileContext(nc) as tc, ExitStack() as ctx:
        const = ctx.enter_context(tc.tile_pool(name="const", bufs=1))
        dram = ctx.enter_context(tc.tile_pool(name="dram", bufs=1, space="DRAM"))
        mm_ps = ctx.enter_context(tc.tile_pool(name="mm_ps", bufs=3, space="PSUM"))
        pv_ps = ctx.enter_context(tc.tile_pool(name="pv_ps", bufs=2, space="PSUM"))
        sm_ps = ctx.enter_context(tc.tile_pool(name="sm_ps", bufs=1, space="PSUM"))
        bc_ps = ctx.enter_context(tc.tile_pool(name="bc_ps", bufs=1, space="PSUM"))

        # ---- resident constants (order = DMA issue order; wkvd first: the
        # first matmuls need it; x^T transposes next; wqd after) ----
        wkvd_sb = const.tile([128, KH, KV_RANK + D_ROPE], bf16, name="wkvd_sb")
        for k in range(KH):
            nc.sync.dma_start(out=wkvd_sb[:, k, :], in_=wkvd_ap[ds(k * 128, 128), :])

        workA = tc.alloc_tile_pool(name="workA", bufs=2)
        xts = {}

        def issue_xt(c):
            xt = workA.tile([128, KH, T], bf16, tag="xt", bufs=2, name=f"xt{c}")
            for i in range(KH):
                nc.sync.dma_start(
                    out=xt[:, i, :],
                    in_=x_ap[ds(c * T, T), ds(i * 128, 128)],
                    transpose=True,
                )
            xts[c] = xt

        issue_xt(0)

        wqd_sb = const.tile([128, KH, Q_RANK], bf16, name="wqd_sb")
        for k in range(KH):
            nc.sync.dma_start(out=wqd_sb[:, k, :], in_=wqd_ap[ds(k * 128, 128), :])
        wqu_sb = const.tile([128, RQ, NH * HEAD_DIM], bf16, name="wqu_sb")
        for r in range(RQ):
            nc.sync.dma_start(out=wqu_sb[:, r, :], in_=wqu_ap[ds(r * 128, 128), :])
        wkvuk_sb = const.tile([128, RKV, NH * D_NOPE], bf16, name="wkvuk_sb")
        for j in range(RKV):
            nc.sync.dma_start(out=wkvuk_sb[:, j, :], in_=wkvuk_ap[ds(j * 128, 128), :])
        wkvuv_sb = const.tile([128, RKV, NH * V_DIM], bf16, name="wkvuv_sb")
        for j in range(RKV):
            nc.sync.dma_start(out=wkvuv_sb[:, j, :], in_=wkvuv_ap[ds(j * 128, 128), :])
        cos_sb = const.tile([128, S], bf16, name="cos_sb")
        nc.sync.dma_start(out=cos_sb[:], in_=cos_ap[:])
        sin_sb = const.tile([128, S], bf16, name="sin_sb")
        nc.sync.dma_start(out=sin_sb[:], in_=sin_ap[:])
        mask_sb = const.tile([128, 1024], bf16, name="mask_sb")
        nc.sync.dma_start(out=mask_sb[:], in_=mask_ap[:])
        onesc_sb = const.tile([128, 1], bf16, name="onesc_sb")
        nc.sync.dma_start(out=onesc_sb[:], in_=onesc_ap[:])
        onesr_f32 = const.tile([1, 128], f32, name="onesr_f32")
        nc.sync.dma_start(out=onesr_f32[:], in_=onesr_ap[:])
        onesr_sb = const.tile([1, 128], dt.float32r, name="onesr_sb")
        with nc.allow_low_precision(reason="exact ones rounded to f32r"):
            nc.vector.tensor_copy(onesr_sb[:], onesr_f32[:])

        # persistent k_rope (rotated, stacked [y1(32); y2(32)])
        krope_sb = const.tile([64, S], bf16, name="krope_sb")
        eps_sb = const.tile([1, 1], f32, name="eps_sb")
        nc.gpsimd.memset(eps_sb[:], EPS)
        # attn^T per head, SBUF-resident through phases B/C
        at_sb = const.tile([128, NH, S], bf16, name="at_sb")

        # DRAM spill buffers
        tk = "ExternalOutput" if taps else "Internal"
        qn_dram = dram.tile([128, NH, S], bf16, name="qn_dram", kind=tk)
        qr_dram = dram.tile([64, NH, S], bf16, name="qr_dram", kind=tk)
        kn_dram = dram.tile([128, NH, S], bf16, name="kn_dram", kind=tk)
        v_dram = dram.tile([128, S // 128, NH * V_DIM], bf16, name="v_dram", kind=tk)
        at_dram = dram.tile([128, NH, S], bf16, name="at_dram", kind=tk) if taps else None

        # ================= phase A: projections =================
        work = workA
        for c in range(NT):
            csl = ds(c * T, T)
            xt = xts.pop(c)
            if c + 1 < NT:
                issue_xt(c + 1)

            # ---- kv down; rmsnorm scalar chain runs under q_down ----
            kvc_bf = work.tile([128, RKV, T], bf16, tag="kvc", bufs=1)
            sq_bf = work.tile([128, RKV, T], bf16, tag="sq", bufs=1)
            for j in range(RKV):
                ps = mm_ps.tile([128, T], f32, tag="mm")
                for k in range(KH):
                    nc.tensor.matmul(
                        ps, wkvd_sb[:, k, ds(j * 128, 128)], xt[:, k, :],
                        start=(k == 0), stop=(k == KH - 1),
                    )
                nc.scalar.activation(sq_bf[:, j, :], ps, AF.Square)
                nc.vector.tensor_copy(kvc_bf[:, j, :], ps)
            ms = sm_ps.tile([1, T], f32, tag="rowps", bufs=2)
            for j in range(RKV):
                nc.tensor.matmul(
                    ms, onesc_sb[:], sq_bf[:, j, :],
                    start=(j == 0), stop=(j == RKV - 1),
                )
            # ---- k rope raw (uses xt; rotation happens later on DVE) ----
            krp = mm_ps.tile([64, T], f32, tag="mm")
            for k in range(KH):
                nc.tensor.matmul(
                    krp, wkvd_sb[:, k, ds(KV_RANK, D_ROPE)], xt[:, k, :],
                    start=(k == 0), stop=(k == KH - 1),
                )
            srt = work.tile([1, T], f32, tag="srt", bufs=1)
            nc.scalar.activation(srt, ms, AF.Sqrt, bias=eps_sb[:], scale=1.0 / KV_RANK)
            rinv = work.tile([1, T], dt.float32r, tag="rinv", bufs=1)
            with nc.allow_low_precision(reason="rsqrt scale rounded to f32r for broadcast matmul"):
                nc.vector.reciprocal(rinv, srt)
            kr_raw = work.tile([64, T], f32, tag="kr_raw", bufs=1)
            nc.vector.tensor_copy(kr_raw, krp)
            kr_sh = work.tile([64, T], f32, tag="kr_sh", bufs=1)
            nc.sync.dma_start(out=kr_sh[0:32, :], in_=kr_raw[32:64, :])
            nc.sync.dma_start(out=kr_sh[32:64, :], in_=kr_raw[0:32, :])
            kt1 = work.tile([64, T], f32, tag="kt1", bufs=1)
            kt2 = work.tile([64, T], f32, tag="kt2", bufs=1)
            nc.vector.tensor_mul(kt1, kr_raw, cos_sb[0:64, csl])
            nc.vector.tensor_mul(kt2, kr_sh, sin_sb[0:64, csl])
            nc.vector.tensor_sub(krope_sb[0:32, csl], kt1[0:32, :], kt2[0:32, :])
            nc.vector.tensor_add(krope_sb[32:64, csl], kt1[32:64, :], kt2[32:64, :])

            # ---- q down: q_lat^T chunk (PE busy while norm chain runs) ----
            qlat = work.tile([128, RQ, T], bf16, tag="qlat", bufs=1)
            for m in range(RQ):
                ps = mm_ps.tile([128, T], f32, tag="mm")
                for k in range(KH):
                    nc.tensor.matmul(
                        ps, wqd_sb[:, k, ds(m * 128, 128)], xt[:, k, :],
                        start=(k == 0), stop=(k == KH - 1),
                    )
                nc.vector.tensor_copy(qlat[:, m, :], ps)

            # ---- kvcn = kvc * rsqrt(ms) (broadcast via K=1 matmul) ----
            rbc_ps = bc_ps.tile([128, T], f32, tag="bc")
            nc.tensor.matmul(rbc_ps, onesr_sb[:], rinv[:], start=True, stop=True)
            rbc = work.tile([128, T], f32, tag="rbc", bufs=1)
            nc.vector.tensor_copy(rbc, rbc_ps)
            kvcn = work.tile([128, RKV, T], bf16, tag="kvcn", bufs=1)
            for j in range(RKV):
                nc.vector.tensor_mul(kvcn[:, j, :], kvc_bf[:, j, :], rbc)

            # ---- q up: nope per head ----
            for m in range(NH):
                ps = mm_ps.tile([128, T], f32, tag="mm")
                for r in range(RQ):
                    nc.tensor.matmul(
                        ps, wqu_sb[:, r, ds(m * 128, 128)], qlat[:, r, :],
                        start=(r == 0), stop=(r == RQ - 1),
                    )
                qnt = work.tile([128, T], bf16, tag="cast", bufs=3)
                nc.vector.tensor_copy(qnt, ps)
                nc.sync.dma_start(out=qn_dram[:, m, csl], in_=qnt)

            # ---- q up: rope (all heads), rotate, scatter per head ----
            ps1 = mm_ps.tile([128, T], f32, tag="mm")
            for r in range(RQ):
                nc.tensor.matmul(
                    ps1, wqu_sb[:, r, ds(NH * D_NOPE, 128)], qlat[:, r, :],
                    start=(r == 0), stop=(r == RQ - 1),
                )
            ps2 = mm_ps.tile([128, T], f32, tag="mm")
            for r in range(RQ):
                nc.tensor.matmul(
                    ps2, wqu_sb[:, r, ds(NH * D_NOPE + 128, 128)], qlat[:, r, :],
                    start=(r == 0), stop=(r == RQ - 1),
                )
            qa = work.tile([128, T], f32, tag="qa", bufs=1)
            qb = work.tile([128, T], f32, tag="qb", bufs=1)
            nc.vector.tensor_mul(qa, ps1, cos_sb[:, csl])
            nc.vector.tensor_mul(qb, ps2, sin_sb[:, csl])
            y1 = work.tile([128, T], bf16, tag="y1", bufs=2)
            nc.vector.tensor_sub(y1, qa, qb)
            qa2 = work.tile([128, T], f32, tag="qa", bufs=1)
            qb2 = work.tile([128, T], f32, tag="qb", bufs=1)
            nc.vector.tensor_mul(qa2, ps2, cos_sb[:, csl])
            nc.vector.tensor_mul(qb2, ps1, sin_sb[:, csl])
            y2 = work.tile([128, T], bf16, tag="y2", bufs=2)
            nc.vector.tensor_add(y2, qa2, qb2)
            for h in range(NH):
                nc.sync.dma_start(out=qr_dram[0:32, h, csl], in_=y1[ds(32 * h, 32), :])
                nc.sync.dma_start(out=qr_dram[32:64, h, csl], in_=y2[ds(32 * h, 32), :])

            # ---- kv up: k_nope^T (feature-major) ----
            for m in range(NH):
                ps = mm_ps.tile([128, T], f32, tag="mm")
                for j in range(RKV):
                    nc.tensor.matmul(
                        ps, wkvuk_sb[:, j, ds(m * 128, 128)], kvcn[:, j, :],
                        start=(j == 0), stop=(j == RKV - 1),
                    )
                knt = work.tile([128, T], bf16, tag="cast", bufs=3)
                nc.vector.tensor_copy(knt, ps)
                nc.sync.dma_start(out=kn_dram[:, m, csl], in_=knt)

            # ---- kv up: v (token-major) ----
            for s2 in range(T // 128):
                ps = mm_ps.tile([128, NH * V_DIM], f32, tag="mm")
                for j in range(RKV):
                    nc.tensor.matmul(
                        ps, kvcn[:, j, ds(s2 * 128, 128)], wkvuv_sb[:, j, :],
                        start=(j == 0), stop=(j == RKV - 1),
                    )
                vt = work.tile([128, NH * V_DIM], bf16, tag="cast", bufs=3)
                nc.vector.tensor_copy(vt, ps)
                nc.sync.dma_start(out=v_dram[:, c * (T // 128) + s2, :], in_=vt)

        # ================= phase B: attention =================
        workA.release()
        workB = tc.alloc_tile_pool(name="workB", bufs=2)
        work = workB

        def drain_norm(st):
            # deferred normalize: by now rec (DVE) has long finished
            h_, qsl_, pv_, rec_ = st
            rb2_ps = bc_ps.tile([128, QC], f32, tag="bc")
            nc.tensor.matmul(rb2_ps, onesr_sb[:], rec_[:], start=True, stop=True)
            rbs = work.tile([128, QC], f32, tag="rbs", bufs=2)
            nc.vector.tensor_copy(rbs, rb2_ps)
            nc.vector.tensor_mul(at_sb[:, h_, qsl_], pv_, rbs)
            if taps:
                nc.sync.dma_start(out=at_dram[:, h_, qsl_], in_=at_sb[:, h_, qsl_])

        norm_pend = []
        for qc in range(NQC):
            qsl = ds(qc * QC, QC)
            nkc = 4 * qc + 4
            for h in range(NH):
                qn_t = work.tile([128, QC], bf16, tag="qn", bufs=2)
                nc.sync.dma_start(out=qn_t, in_=qn_dram[:, h, qsl])
                qr_t = work.tile([64, QC], bf16, tag="qr", bufs=2)
                nc.sync.dma_start(out=qr_t, in_=qr_dram[:, h, qsl])
                pv = pv_ps.tile([128, QC], f32, tag="pv")
                den = sm_ps.tile([1, QC], f32, tag="rowps", bufs=2)
                pend = []
                for kc in range(nkc):
                    kn_t = work.tile([128, 128], bf16, tag="kn", bufs=10)
                    nc.sync.dma_start(out=kn_t, in_=kn_dram[:, h, ds(kc * 128, 128)])
                    v_t = work.tile([128, 128], bf16, tag="vt", bufs=10)
                    nc.sync.dma_start(out=v_t, in_=v_dram[:, kc, ds(h * V_DIM, V_DIM)])
                    sps = mm_ps.tile([128, QC], f32, tag="mm")
                    nc.tensor.matmul(sps, kn_t, qn_t, start=True, stop=False)
                    nc.tensor.matmul(
                        sps, krope_sb[:, ds(kc * 128, 128)], qr_t,
                        start=False, stop=True,
                    )
                    E = work.tile([128, QC], bf16, tag="E", bufs=8)
                    nc.scalar.activation(E, sps, AF.Exp, scale=SCALE)
                    dm = kc - 4 * qc
                    if dm >= 0:
                        nc.vector.tensor_mul(E, E, mask_sb[:, ds(512 - 128 * dm, 512)])
                    pend.append((kc, E, v_t))
                    if len(pend) > 3:  # drain den/pv three kc behind the scores
                        pkc, pE, pvt = pend.pop(0)
                        nc.tensor.matmul(den, onesc_sb[:], pE, start=(pkc == 0), stop=False)
                        nc.tensor.matmul(pv, pvt, pE, start=(pkc == 0), stop=False)
                while pend:
                    last = len(pend) == 1
                    pkc, pE, pvt = pend.pop(0)
                    nc.tensor.matmul(den, onesc_sb[:], pE, start=(pkc == 0), stop=last)
                    nc.tensor.matmul(pv, pvt, pE, start=(pkc == 0), stop=last)
                rec = work.tile([1, QC], dt.float32r, tag="rec", bufs=2)
                with nc.allow_low_precision(reason="softmax denom rounded to f32r for broadcast matmul"):
                    nc.vector.reciprocal(rec, den)
                norm_pend.append((h, qsl, pv, rec))
                if len(norm_pend) > 1:
                    drain_norm(norm_pend.pop(0))
        while norm_pend:
            drain_norm(norm_pend.pop(0))

        # ================= phase C: out-projection =================
        workB.release()
        workC = ctx.enter_context(tc.tile_pool(name="workC", bufs=2))
        work = workC
        for n in range(HID // 512):
            wo_t = work.tile([128, NH, 512], bf16, tag="wo", bufs=2)
            for f in range(NH):
                nc.sync.dma_start(
                    out=wo_t[:, f, :], in_=wout_ap[ds(f * 128, 128), ds(n * 512, 512)]
                )
            for t16 in range(S // 128):
                ps = mm_ps.tile([128, 512], f32, tag="mm")
                for f in range(NH):
                    nc.tensor.matmul(
                        ps, at_sb[:, f, ds(t16 * 128, 128)], wo_t[:, f, :],
                        start=(f == 0), stop=(f == NH - 1),
                    )
                o_t = work.tile([128, 512], f32, tag="ot", bufs=3)
                nc.vector.tensor_copy(o_t, ps)
                nc.sync.dma_start(
                    out=out_ap[ds(t16 * 128, 128), ds(n * 512, 512)], in_=o_t
                )

    nc.compile()
    return nc


def get_nc():
    if "nc" not in _CACHE:
        _CACHE["nc"] = build_nc()
    return _CACHE["nc"]


def host_inputs(x, w_q_down, w_q_up, w_kv_down, kv_norm_w, w_kv_up, w_out):
    """Build the 8 per-core input shards (host-side prep, numpy only)."""
    bf = ml_dtypes.bfloat16
    x = np.asarray(x, np.float32)
    inv = 1.0 / THETA ** (np.arange(0, D_ROPE, 2, dtype=np.float64) / D_ROPE)
    ang = np.arange(S, dtype=np.float64)[:, None] * inv[None, :]      # (S, 32)
    cosq = np.ascontiguousarray(np.tile(np.cos(ang).T, (4, 1))).astype(bf)  # (128, S)
    sinq = np.ascontiguousarray(np.tile(np.sin(ang).T, (4, 1))).astype(bf)
    maskt = (
        np.arange(1024)[None, :] >= (np.arange(128)[:, None] + 512)
    ).astype(bf)
    ones_col = np.ones((128, 1), bf)
    ones_row = np.ones((1, 128), np.float32)
    wkv_eff = np.asarray(w_kv_up, np.float32) * np.asarray(kv_norm_w, np.float32)[:, None]

    x_bf = [np.ascontiguousarray(x[b]).astype(bf) for b in range(B)]
    wqd_bf = np.asarray(w_q_down, np.float32).astype(bf)
    wkvd_bf = np.asarray(w_kv_down, np.float32).astype(bf)
    wqu_f = np.asarray(w_q_up, np.float32)
    wout_f = np.asarray(w_out, np.float32)

    in_maps = []
    for ci in range(NCORES):
        b, hg = divmod(ci, 4)
        heads = list(range(NH * hg, NH * hg + NH))
        qu_cols = (
            [h * HEAD_DIM + j for h in heads for j in range(D_NOPE)]
            + [h * HEAD_DIM + D_NOPE + j for h in heads for j in range(32)]
            + [h * HEAD_DIM + D_NOPE + 32 + j for h in heads for j in range(32)]
        )
        kn_cols = [h * (D_NOPE + V_DIM) + j for h in heads for j in range(D_NOPE)]
        v_cols = [h * (D_NOPE + V_DIM) + D_NOPE + j for h in heads for j in range(V_DIM)]
        in_maps.append(
            {
                "x": x_bf[b],
                "wqd": wqd_bf,
                "wqu": np.ascontiguousarray(wqu_f[:, qu_cols]).astype(bf),
                "wkvd": wkvd_bf,
                "wkvuk": np.ascontiguousarray(wkv_eff[:, kn_cols]).astype(bf),
                "wkvuv": np.ascontiguousarray(wkv_eff[:, v_cols]).astype(bf),
                "wout": np.ascontiguousarray(
                    wout_f[NH * V_DIM * hg : NH * V_DIM * (hg + 1), :]
                ).astype(bf),
                "cosq": cosq,
                "sinq": sinq,
                "maskt": maskt,
                "ones_col": ones_col,
                "ones_row": ones_row,
            }
        )
    return in_maps


def run(inputs, trace=False, trace_cores=None):
    from concourse.bass_utils import run_bass_kernel_spmd

    nc = get_nc()
    in_maps = host_inputs(**inputs)
    res = run_bass_kernel_spmd(
        nc,
        in_maps,
        core_ids=list(range(NCORES)),
        trace=trace,
        trace_cores=trace_cores,
    )
    out = np.zeros((B, S, HID), np.float32)
    for ci in range(NCORES):
        out[ci // 4] += res.results[ci]["out"]
    return out, res


def kernel(**inputs):
    out, _ = run(inputs, trace=False)
    return out


# revision 18
# speedup vs baseline: 1.0115x; 1.0115x over previous
"""MLA attention kernel for Trainium2 — 8-core tensor-parallel (self-contained).

Sharding: data-parallel over batch (2) x tensor-parallel over head groups
(4 groups of 4 heads) = 8 cores, SPMD (one NEFF, per-core input shards).
Core ci: batch ci//4, heads [4*(ci%4), 4*(ci%4)+4).

Per-core dataflow (everything feature-major "transposed" so the PE never
needs an on-chip transpose):
  x^T tiles via bf16 DMA-transpose (prefetched one chunk ahead)
  kv^T = wkvd.T @ x^T (rmsnorm sum via ones-matmul, scale broadcast via K=1 matmul)
  q_lat^T = wqd.T @ x^T ; q^T = wqu.T @ q_lat^T ; k_nope^T = wkvuk.T @ kv_c^T
  v (token-major) = kv_c^T.T @ wkvuv
  RoPE on rope rows (DVE); host-permuted weight columns group x1/x2 rows
  scores^T[k,q] = k^T.T @ q^T -> exp (ACT, scale folded) -> causal mask (DVE)
  denom[1,q] = ones.T @ E ; out_h^T[v,q] = v.T @ E   (both pipelined 3 deep)
  normalize via K=1 broadcast matmul of 1/denom, deferred one iteration
  out[t, hid] = attn^T.T @ w_out (token-major, contiguous writes)
Host: sums the 4 partial outputs per batch.
"""

import math

import numpy as np
import ml_dtypes

# ---- problem constants (from the reference model) ----
B, S, HID = 2, 2048, 2048
H, D_NOPE, D_ROPE, V_DIM = 16, 128, 64, 128
KV_RANK, Q_RANK = 512, 1536
HEAD_DIM = D_NOPE + D_ROPE
THETA, EPS = 10000.0, 1e-6
NCORES = 8
NH = 4                    # heads per core
T = 512                   # phase-A token chunk
NT = S // T
QC = 512                  # attention query chunk
NQC = S // QC
KH = HID // 128           # 16 k-chunks over HID
RQ = Q_RANK // 128        # 12 chunks over q rank
RKV = KV_RANK // 128      # 4 chunks over kv rank
SCALE = 1.0 / math.sqrt(HEAD_DIM)

_CACHE = {}


def build_nc(taps=False):
    """Build the Bass/Tile program (one NeuronCore, run SPMD on 8)."""
    from contextlib import ExitStack

    import concourse.mybir as mybir
    import concourse.tile as tile
    from concourse import bacc
    from concourse.bass import ds

    dt = mybir.dt
    AF = mybir.ActivationFunctionType
    bf16 = dt.bfloat16
    f32 = dt.float32

    nc = bacc.Bacc(
        "TRN2",
        target_bir_lowering=False,
        debug=False,
        enable_asserts=False,
        num_devices=NCORES,
    )

    # ---- I/O ----
    x_ap = nc.dram_tensor("x", [S, HID], bf16, kind="ExternalInput").ap()
    wqd_ap = nc.dram_tensor("wqd", [HID, Q_RANK], bf16, kind="ExternalInput").ap()
    wqu_ap = nc.dram_tensor("wqu", [Q_RANK, NH * HEAD_DIM], bf16, kind="ExternalInput").ap()
    wkvd_ap = nc.dram_tensor("wkvd", [HID, KV_RANK + D_ROPE], bf16, kind="ExternalInput").ap()
    wkvuk_ap = nc.dram_tensor("wkvuk", [KV_RANK, NH * D_NOPE], bf16, kind="ExternalInput").ap()
    wkvuv_ap = nc.dram_tensor("wkvuv", [KV_RANK, NH * V_DIM], bf16, kind="ExternalInput").ap()
    wout_ap = nc.dram_tensor("wout", [NH * V_DIM, HID], bf16, kind="ExternalInput").ap()
    cos_ap = nc.dram_tensor("cosq", [128, S], bf16, kind="ExternalInput").ap()
    sin_ap = nc.dram_tensor("sinq", [128, S], bf16, kind="ExternalInput").ap()
    mask_ap = nc.dram_tensor("maskt", [128, 1024], bf16, kind="ExternalInput").ap()
    onesc_ap = nc.dram_tensor("ones_col", [128, 1], bf16, kind="ExternalInput").ap()
    onesr_ap = nc.dram_tensor("ones_row", [1, 128], f32, kind="ExternalInput").ap()
    out_ap = nc.dram_tensor("out", [S, HID], f32, kind="ExternalOutput").ap()

    with tile.TileContext(nc) as tc, ExitStack() as ctx:
        const = ctx.enter_context(tc.tile_pool(name="const", bufs=1))
        dram = ctx.enter_context(tc.tile_pool(name="dram", bufs=1, space="DRAM"))
        mm_ps = ctx.enter_context(tc.tile_pool(name="mm_ps", bufs=3, space="PSUM"))
        pv_ps = ctx.enter_context(tc.tile_pool(name="pv_ps", bufs=2, space="PSUM"))
        sm_ps = ctx.enter_context(tc.tile_pool(name="sm_ps", bufs=1, space="PSUM"))
        bc_ps = ctx.enter_context(tc.tile_pool(name="bc_ps", bufs=1, space="PSUM"))

        # ---- resident constants (order = DMA issue order; wkvd first: the
        # first matmuls need it; x^T transposes next; wqd after) ----
        workA = tc.alloc_tile_pool(name="workA", bufs=2)
        xts = {}

        def issue_xt(c):
            xt = workA.tile([128, KH, T], bf16, tag="xt", bufs=2, name=f"xt{c}")
            for i in range(KH):
                nc.sync.dma_start(
                    out=xt[:, i, :],
                    in_=x_ap[ds(c * T, T), ds(i * 128, 128)],
                    transpose=True,
                )
            xts[c] = xt

        issue_xt(0)

        wkvd_sb = const.tile([128, KH, KV_RANK + D_ROPE], bf16, name="wkvd_sb")
        for k in range(KH):
            nc.sync.dma_start(out=wkvd_sb[:, k, :], in_=wkvd_ap[ds(k * 128, 128), :])
        wqd_sb = const.tile([128, KH, Q_RANK], bf16, name="wqd_sb")
        for k in range(KH):
            nc.sync.dma_start(out=wqd_sb[:, k, :], in_=wqd_ap[ds(k * 128, 128), :])
        wqu_sb = const.tile([128, RQ, NH * HEAD_DIM], bf16, name="wqu_sb")
        for r in range(RQ):
            nc.sync.dma_start(out=wqu_sb[:, r, :], in_=wqu_ap[ds(r * 128, 128), :])
        wkvuk_sb = const.tile([128, RKV, NH * D_NOPE], bf16, name="wkvuk_sb")
        for j in range(RKV):
            nc.sync.dma_start(out=wkvuk_sb[:, j, :], in_=wkvuk_ap[ds(j * 128, 128), :])
        wkvuv_sb = const.tile([128, RKV, NH * V_DIM], bf16, name="wkvuv_sb")
        for j in range(RKV):
            nc.sync.dma_start(out=wkvuv_sb[:, j, :], in_=wkvuv_ap[ds(j * 128, 128), :])
        cos_sb = const.tile([128, S], bf16, name="cos_sb")
        nc.sync.dma_start(out=cos_sb[:], in_=cos_ap[:])
        sin_sb = const.tile([128, S], bf16, name="sin_sb")
        nc.sync.dma_start(out=sin_sb[:], in_=sin_ap[:])
        mask_sb = const.tile([128, 1024], bf16, name="mask_sb")
        nc.sync.dma_start(out=mask_sb[:], in_=mask_ap[:])
        onesc_sb = const.tile([128, 1], bf16, name="onesc_sb")
        nc.sync.dma_start(out=onesc_sb[:], in_=onesc_ap[:])
        onesr_f32 = const.tile([1, 128], f32, name="onesr_f32")
        nc.sync.dma_start(out=onesr_f32[:], in_=onesr_ap[:])
        onesr_sb = const.tile([1, 128], dt.float32r, name="onesr_sb")
        with nc.allow_low_precision(reason="exact ones rounded to f32r"):
            nc.vector.tensor_copy(onesr_sb[:], onesr_f32[:])

        # persistent k_rope (rotated, stacked [y1(32); y2(32)])
        krope_sb = const.tile([64, S], bf16, name="krope_sb")
        eps_sb = const.tile([1, 1], f32, name="eps_sb")
        nc.gpsimd.memset(eps_sb[:], EPS)
        # attn^T per head, SBUF-resident through phases B/C
        at_sb = const.tile([128, NH, S], bf16, name="at_sb")

        # DRAM spill buffers
        tk = "ExternalOutput" if taps else "Internal"
        qn_dram = dram.tile([128, NH, S], bf16, name="qn_dram", kind=tk)
        qr_dram = dram.tile([64, NH, S], bf16, name="qr_dram", kind=tk)
        kn_dram = dram.tile([128, NH, S], bf16, name="kn_dram", kind=tk)
        v_dram = dram.tile([128, S // 128, NH * V_DIM], bf16, name="v_dram", kind=tk)
        at_dram = dram.tile([128, NH, S], bf16, name="at_dram", kind=tk) if taps else None

        # ================= phase A: projections =================
        work = workA
        for c in range(NT):
            csl = ds(c * T, T)
            xt = xts.pop(c)
            if c + 1 < NT:
                issue_xt(c + 1)

            # ---- kv down; rmsnorm scalar chain runs under q_down ----
            kvc_bf = work.tile([128, RKV, T], bf16, tag="kvc", bufs=1)
            sq_bf = work.tile([128, RKV, T], bf16, tag="sq", bufs=1)
            for j in range(RKV):
                ps = mm_ps.tile([128, T], f32, tag="mm")
                for k in range(KH):
                    nc.tensor.matmul(
                        ps, wkvd_sb[:, k, ds(j * 128, 128)], xt[:, k, :],
                        start=(k == 0), stop=(k == KH - 1),
                    )
                nc.scalar.activation(sq_bf[:, j, :], ps, AF.Square)
                nc.vector.tensor_copy(kvc_bf[:, j, :], ps)
            ms = sm_ps.tile([1, T], f32, tag="rowps", bufs=2)
            for j in range(RKV):
                nc.tensor.matmul(
                    ms, onesc_sb[:], sq_bf[:, j, :],
                    start=(j == 0), stop=(j == RKV - 1),
                )
            # ---- k rope raw (uses xt; rotation happens later on DVE) ----
            krp = mm_ps.tile([64, T], f32, tag="mm")
            for k in range(KH):
                nc.tensor.matmul(
                    krp, wkvd_sb[:, k, ds(KV_RANK, D_ROPE)], xt[:, k, :],
                    start=(k == 0), stop=(k == KH - 1),
                )
            srt = work.tile([1, T], f32, tag="srt", bufs=1)
            nc.scalar.activation(srt, ms, AF.Sqrt, bias=eps_sb[:], scale=1.0 / KV_RANK)
            rinv = work.tile([1, T], dt.float32r, tag="rinv", bufs=1)
            with nc.allow_low_precision(reason="rsqrt scale rounded to f32r for broadcast matmul"):
                nc.vector.reciprocal(rinv, srt)
            kr_raw = work.tile([64, T], f32, tag="kr_raw", bufs=1)
            nc.vector.tensor_copy(kr_raw, krp)
            kr_sh = work.tile([64, T], f32, tag="kr_sh", bufs=1)
            nc.gpsimd.dma_start(out=kr_sh[0:32, :], in_=kr_raw[32:64, :])
            nc.gpsimd.dma_start(out=kr_sh[32:64, :], in_=kr_raw[0:32, :])
            kt1 = work.tile([64, T], f32, tag="kt1", bufs=1)
            kt2 = work.tile([64, T], f32, tag="kt2", bufs=1)
            nc.vector.tensor_mul(kt1, kr_raw, cos_sb[0:64, csl])
            nc.vector.tensor_mul(kt2, kr_sh, sin_sb[0:64, csl])
            nc.vector.tensor_sub(krope_sb[0:32, csl], kt1[0:32, :], kt2[0:32, :])
            nc.vector.tensor_add(krope_sb[32:64, csl], kt1[32:64, :], kt2[32:64, :])

            # ---- q down: q_lat^T chunk (PE busy while norm chain runs) ----
            qlat = work.tile([128, RQ, T], bf16, tag="qlat", bufs=1)
            for m in range(RQ):
                ps = mm_ps.tile([128, T], f32, tag="mm")
                for k in range(KH):
                    nc.tensor.matmul(
                        ps, wqd_sb[:, k, ds(m * 128, 128)], xt[:, k, :],
                        start=(k == 0), stop=(k == KH - 1),
                    )
                nc.vector.tensor_copy(qlat[:, m, :], ps)

            # ---- kvcn = kvc * rsqrt(ms) (broadcast via K=1 matmul) ----
            rbc_ps = bc_ps.tile([128, T], f32, tag="bc")
            nc.tensor.matmul(rbc_ps, onesr_sb[:], rinv[:], start=True, stop=True)
            rbc = work.tile([128, T], f32, tag="rbc", bufs=1)
            nc.vector.tensor_copy(rbc, rbc_ps)
            kvcn = work.tile([128, RKV, T], bf16, tag="kvcn", bufs=1)
            for j in range(RKV):
                nc.vector.tensor_mul(kvcn[:, j, :], kvc_bf[:, j, :], rbc)

            # ---- q up: nope per head ----
            for m in range(NH):
                ps = mm_ps.tile([128, T], f32, tag="mm")
                for r in range(RQ):
                    nc.tensor.matmul(
                        ps, wqu_sb[:, r, ds(m * 128, 128)], qlat[:, r, :],
                        start=(r == 0), stop=(r == RQ - 1),
                    )
                qnt = work.tile([128, T], bf16, tag="cast", bufs=3)
                nc.vector.tensor_copy(qnt, ps)
                nc.gpsimd.dma_start(out=qn_dram[:, m, csl], in_=qnt)

            # ---- q up: rope (all heads), rotate, scatter per head ----
            ps1 = mm_ps.tile([128, T], f32, tag="mm")
            for r in range(RQ):
                nc.tensor.matmul(
                    ps1, wqu_sb[:, r, ds(NH * D_NOPE, 128)], qlat[:, r, :],
                    start=(r == 0), stop=(r == RQ - 1),
                )
            ps2 = mm_ps.tile([128, T], f32, tag="mm")
            for r in range(RQ):
                nc.tensor.matmul(
                    ps2, wqu_sb[:, r, ds(NH * D_NOPE + 128, 128)], qlat[:, r, :],
                    start=(r == 0), stop=(r == RQ - 1),
                )
            qa = work.tile([128, T], f32, tag="qa", bufs=1)
            qb = work.tile([128, T], f32, tag="qb", bufs=1)
            nc.vector.tensor_mul(qa, ps1, cos_sb[:, csl])
            nc.vector.tensor_mul(qb, ps2, sin_sb[:, csl])
            y1 = work.tile([128, T], bf16, tag="y1", bufs=2)
            nc.vector.tensor_sub(y1, qa, qb)
            qa2 = work.tile([128, T], f32, tag="qa", bufs=1)
            qb2 = work.tile([128, T], f32, tag="qb", bufs=1)
            nc.vector.tensor_mul(qa2, ps2, cos_sb[:, csl])
            nc.vector.tensor_mul(qb2, ps1, sin_sb[:, csl])
            y2 = work.tile([128, T], bf16, tag="y2", bufs=2)
            nc.vector.tensor_add(y2, qa2, qb2)
            for h in range(NH):
                nc.gpsimd.dma_start(out=qr_dram[0:32, h, csl], in_=y1[ds(32 * h, 32), :])
                nc.gpsimd.dma_start(out=qr_dram[32:64, h, csl], in_=y2[ds(32 * h, 32), :])

            # ---- kv up: k_nope^T (feature-major) ----
            for m in range(NH):
                ps = mm_ps.tile([128, T], f32, tag="mm")
                for j in range(RKV):
                    nc.tensor.matmul(
                        ps, wkvuk_sb[:, j, ds(m * 128, 128)], kvcn[:, j, :],
                        start=(j == 0), stop=(j == RKV - 1),
                    )
                knt = work.tile([128, T], bf16, tag="cast", bufs=3)
                nc.vector.tensor_copy(knt, ps)
                nc.gpsimd.dma_start(out=kn_dram[:, m, csl], in_=knt)

            # ---- kv up: v (token-major) ----
            for s2 in range(T // 128):
                ps = mm_ps.tile([128, NH * V_DIM], f32, tag="mm")
                for j in range(RKV):
                    nc.tensor.matmul(
                        ps, kvcn[:, j, ds(s2 * 128, 128)], wkvuv_sb[:, j, :],
                        start=(j == 0), stop=(j == RKV - 1),
                    )
                vt = work.tile([128, NH * V_DIM], bf16, tag="cast", bufs=3)
                nc.vector.tensor_copy(vt, ps)
                nc.gpsimd.dma_start(out=v_dram[:, c * (T // 128) + s2, :], in_=vt)

        # ================= phase B: attention =================
        workA.release()
        workB = tc.alloc_tile_pool(name="workB", bufs=2)
        work = workB

        def drain_norm(st):
            # deferred normalize: by now rec (DVE) has long finished
            h_, qsl_, pv_, rec_ = st
            rb2_ps = bc_ps.tile([128, QC], f32, tag="bc")
            nc.tensor.matmul(rb2_ps, onesr_sb[:], rec_[:], start=True, stop=True)
            rbs = work.tile([128, QC], f32, tag="rbs", bufs=2)
            nc.vector.tensor_copy(rbs, rb2_ps)
            nc.vector.tensor_mul(at_sb[:, h_, qsl_], pv_, rbs)
            if taps:
                nc.sync.dma_start(out=at_dram[:, h_, qsl_], in_=at_sb[:, h_, qsl_])

        norm_pend = []
        for qc in range(NQC):
            qsl = ds(qc * QC, QC)
            nkc = 4 * qc + 4
            for h in range(NH):
                qn_t = work.tile([128, QC], bf16, tag="qn", bufs=2)
                nc.sync.dma_start(out=qn_t, in_=qn_dram[:, h, qsl])
                qr_t = work.tile([64, QC], bf16, tag="qr", bufs=2)
                nc.sync.dma_start(out=qr_t, in_=qr_dram[:, h, qsl])
                pv = pv_ps.tile([128, QC], f32, tag="pv")
                den = sm_ps.tile([1, QC], f32, tag="rowps", bufs=2)
                pend = []
                for kc in range(nkc):
                    kn_t = work.tile([128, 128], bf16, tag="kn", bufs=12)
                    nc.sync.dma_start(out=kn_t, in_=kn_dram[:, h, ds(kc * 128, 128)])
                    v_t = work.tile([128, 128], bf16, tag="vt", bufs=12)
                    nc.sync.dma_start(out=v_t, in_=v_dram[:, kc, ds(h * V_DIM, V_DIM)])
                    sps = mm_ps.tile([128, QC], f32, tag="mm")
                    nc.tensor.matmul(sps, kn_t, qn_t, start=True, stop=False)
                    nc.tensor.matmul(
                        sps, krope_sb[:, ds(kc * 128, 128)], qr_t,
                        start=False, stop=True,
                    )
                    E = work.tile([128, QC], bf16, tag="E", bufs=9)
                    nc.scalar.activation(E, sps, AF.Exp, scale=SCALE)
                    dm = kc - 4 * qc
                    if dm >= 0:
                        nc.vector.tensor_mul(E, E, mask_sb[:, ds(512 - 128 * dm, 512)])
                    pend.append((kc, E, v_t))
                    if len(pend) > 6:  # drain den/pv six kc behind the scores
                        pkc, pE, pvt = pend.pop(0)
                        nc.tensor.matmul(den, onesc_sb[:], pE, start=(pkc == 0), stop=False)
                        nc.tensor.matmul(pv, pvt, pE, start=(pkc == 0), stop=False)
                while pend:
                    last = len(pend) == 1
                    pkc, pE, pvt = pend.pop(0)
                    nc.tensor.matmul(den, onesc_sb[:], pE, start=(pkc == 0), stop=last)
                    nc.tensor.matmul(pv, pvt, pE, start=(pkc == 0), stop=last)
                rec = work.tile([1, QC], dt.float32r, tag="rec", bufs=2)
                with nc.allow_low_precision(reason="softmax denom rounded to f32r for broadcast matmul"):
                    nc.vector.reciprocal(rec, den)
                norm_pend.append((h, qsl, pv, rec))
                if len(norm_pend) > 1:
                    drain_norm(norm_pend.pop(0))
        while norm_pend:
            drain_norm(norm_pend.pop(0))

        # ================= phase C: out-projection =================
        workB.release()
        workC = ctx.enter_context(tc.tile_pool(name="workC", bufs=2))
        work = workC
        wo_ts = []
        for n in range(HID // 512):
            wo_t = work.tile([128, NH, 512], bf16, tag="wo", bufs=4)
            for f in range(NH):
                nc.sync.dma_start(
                    out=wo_t[:, f, :], in_=wout_ap[ds(f * 128, 128), ds(n * 512, 512)]
                )
            wo_ts.append(wo_t)
        for n in range(HID // 512):
            wo_t = wo_ts[n]
            for t16 in range(S // 128):
                ps = mm_ps.tile([128, 512], f32, tag="mm")
                for f in range(NH):
                    nc.tensor.matmul(
                        ps, at_sb[:, f, ds(t16 * 128, 128)], wo_t[:, f, :],
                        start=(f == 0), stop=(f == NH - 1),
                    )
                o_t = work.tile([128, 512], f32, tag="ot", bufs=3)
                nc.vector.tensor_copy(o_t, ps)
                nc.sync.dma_start(
                    out=out_ap[ds(t16 * 128, 128), ds(n * 512, 512)], in_=o_t
                )

    nc.compile()
    return nc


def get_nc():
    if "nc" not in _CACHE:
        _CACHE["nc"] = build_nc()
    return _CACHE["nc"]


def host_inputs(x, w_q_down, w_q_up, w_kv_down, kv_norm_w, w_kv_up, w_out):
    """Build the 8 per-core input shards (host-side prep, numpy only)."""
    bf = ml_dtypes.bfloat16
    x = np.asarray(x, np.float32)
    inv = 1.0 / THETA ** (np.arange(0, D_ROPE, 2, dtype=np.float64) / D_ROPE)
    ang = np.arange(S, dtype=np.float64)[:, None] * inv[None, :]      # (S, 32)
    cosq = np.ascontiguousarray(np.tile(np.cos(ang).T, (4, 1))).astype(bf)  # (128, S)
    sinq = np.ascontiguousarray(np.tile(np.sin(ang).T, (4, 1))).astype(bf)
    maskt = (
        np.arange(1024)[None, :] >= (np.arange(128)[:, None] + 512)
    ).astype(bf)
    ones_col = np.ones((128, 1), bf)
    ones_row = np.ones((1, 128), np.float32)
    wkv_eff = np.asarray(w_kv_up, np.float32) * np.asarray(kv_norm_w, np.float32)[:, None]

    x_bf = [np.ascontiguousarray(x[b]).astype(bf) for b in range(B)]
    wqd_bf = np.asarray(w_q_down, np.float32).astype(bf)
    wkvd_bf = np.asarray(w_kv_down, np.float32).astype(bf)
    wqu_f = np.asarray(w_q_up, np.float32)
    wout_f = np.asarray(w_out, np.float32)

    in_maps = []
    for ci in range(NCORES):
        b, hg = divmod(ci, 4)
        heads = list(range(NH * hg, NH * hg + NH))
        qu_cols = (
            [h * HEAD_DIM + j for h in heads for j in range(D_NOPE)]
            + [h * HEAD_DIM + D_NOPE + j for h in heads for j in range(32)]
            + [h * HEAD_DIM + D_NOPE + 32 + j for h in heads for j in range(32)]
        )
        kn_cols = [h * (D_NOPE + V_DIM) + j for h in heads for j in range(D_NOPE)]
        v_cols = [h * (D_NOPE + V_DIM) + D_NOPE + j for h in heads for j in range(V_DIM)]
        in_maps.append(
            {
                "x": x_bf[b],
                "wqd": wqd_bf,
                "wqu": np.ascontiguousarray(wqu_f[:, qu_cols]).astype(bf),
                "wkvd": wkvd_bf,
                "wkvuk": np.ascontiguousarray(wkv_eff[:, kn_cols]).astype(bf),
                "wkvuv": np.ascontiguousarray(wkv_eff[:, v_cols]).astype(bf),
                "wout": np.ascontiguousarray(
                    wout_f[NH * V_DIM * hg : NH * V_DIM * (hg + 1), :]
                ).astype(bf),
                "cosq": cosq,
                "sinq": sinq,
                "maskt": maskt,
                "ones_col": ones_col,
                "ones_row": ones_row,
            }
        )
    return in_maps


def run(inputs, trace=False, trace_cores=None):
    from concourse.bass_utils import run_bass_kernel_spmd

    nc = get_nc()
    in_maps = host_inputs(**inputs)
    res = run_bass_kernel_spmd(
        nc,
        in_maps,
        core_ids=list(range(NCORES)),
        trace=trace,
        trace_cores=trace_cores,
    )
    out = np.zeros((B, S, HID), np.float32)
    for ci in range(NCORES):
        out[ci // 4] += res.results[ci]["out"]
    return out, res


def kernel(**inputs):
    out, _ = run(inputs, trace=False)
    return out


# revision 19
# speedup vs baseline: 1.0611x; 1.0490x over previous
"""MLA attention kernel for Trainium2 — 8-core tensor-parallel (self-contained).

Sharding: data-parallel over batch (2) x tensor-parallel over head groups
(4 groups of 4 heads) = 8 cores, SPMD (one NEFF, per-core input shards).
Core ci: batch ci//4, heads [4*(ci%4), 4*(ci%4)+4).

Per-core dataflow (everything feature-major "transposed" so the PE never
needs an on-chip transpose):
  x^T tiles via bf16 DMA-transpose (prefetched one chunk ahead)
  kv^T = wkvd.T @ x^T (rmsnorm sum via ones-matmul, scale broadcast via K=1 matmul)
  q_lat^T = wqd.T @ x^T ; q^T = wqu.T @ q_lat^T ; k_nope^T = wkvuk.T @ kv_c^T
  v (token-major) = kv_c^T.T @ wkvuv
  RoPE on rope rows (DVE); host-permuted weight columns group x1/x2 rows
  scores^T[k,q] = k^T.T @ q^T -> exp (ACT, scale folded) -> causal mask (DVE)
  denom[1,q] = ones.T @ E ; out_h^T[v,q] = v.T @ E   (both pipelined 3 deep)
  normalize via K=1 broadcast matmul of 1/denom, deferred one iteration
  out[t, hid] = attn^T.T @ w_out (token-major, contiguous writes)
Host: sums the 4 partial outputs per batch.
"""

import math

import numpy as np
import ml_dtypes

# ---- problem constants (from the reference model) ----
B, S, HID = 2, 2048, 2048
H, D_NOPE, D_ROPE, V_DIM = 16, 128, 64, 128
KV_RANK, Q_RANK = 512, 1536
HEAD_DIM = D_NOPE + D_ROPE
THETA, EPS = 10000.0, 1e-6
NCORES = 8
NH = 4                    # heads per core
T = 512                   # phase-A token chunk
NT = S // T
QC = 512                  # attention query chunk
NQC = S // QC
KH = HID // 128           # 16 k-chunks over HID
RQ = Q_RANK // 128        # 12 chunks over q rank
RKV = KV_RANK // 128      # 4 chunks over kv rank
SCALE = 1.0 / math.sqrt(HEAD_DIM)

_CACHE = {}


def build_nc(taps=False):
    """Build the Bass/Tile program (one NeuronCore, run SPMD on 8)."""
    from contextlib import ExitStack

    import concourse.mybir as mybir
    import concourse.tile as tile
    from concourse import bacc
    from concourse.bass import ds

    dt = mybir.dt
    AF = mybir.ActivationFunctionType
    bf16 = dt.bfloat16
    f32 = dt.float32

    nc = bacc.Bacc(
        "TRN2",
        target_bir_lowering=False,
        debug=False,
        enable_asserts=False,
        num_devices=NCORES,
    )

    # ---- I/O ----
    x_ap = nc.dram_tensor("x", [HID, S], bf16, kind="ExternalInput").ap()
    wqd_ap = nc.dram_tensor("wqd", [HID, Q_RANK], bf16, kind="ExternalInput").ap()
    wqu_ap = nc.dram_tensor("wqu", [Q_RANK, NH * HEAD_DIM], bf16, kind="ExternalInput").ap()
    wkvd_ap = nc.dram_tensor("wkvd", [HID, KV_RANK + D_ROPE], bf16, kind="ExternalInput").ap()
    wkvuk_ap = nc.dram_tensor("wkvuk", [KV_RANK, NH * D_NOPE], bf16, kind="ExternalInput").ap()
    wkvuv_ap = nc.dram_tensor("wkvuv", [KV_RANK, NH * V_DIM], bf16, kind="ExternalInput").ap()
    wout_ap = nc.dram_tensor("wout", [NH * V_DIM, HID], bf16, kind="ExternalInput").ap()
    cos_ap = nc.dram_tensor("cosq", [128, S], bf16, kind="ExternalInput").ap()
    sin_ap = nc.dram_tensor("sinq", [128, S], bf16, kind="ExternalInput").ap()
    mask_ap = nc.dram_tensor("maskt", [128, 1024], bf16, kind="ExternalInput").ap()
    onesc_ap = nc.dram_tensor("ones_col", [128, 1], bf16, kind="ExternalInput").ap()
    onesr_ap = nc.dram_tensor("ones_row", [1, 128], f32, kind="ExternalInput").ap()
    out_ap = nc.dram_tensor("out", [S, HID], f32, kind="ExternalOutput").ap()

    with tile.TileContext(nc) as tc, ExitStack() as ctx:
        const = ctx.enter_context(tc.tile_pool(name="const", bufs=1))
        dram = ctx.enter_context(tc.tile_pool(name="dram", bufs=1, space="DRAM"))
        mm_ps = ctx.enter_context(tc.tile_pool(name="mm_ps", bufs=3, space="PSUM"))
        pv_ps = ctx.enter_context(tc.tile_pool(name="pv_ps", bufs=2, space="PSUM"))
        sm_ps = ctx.enter_context(tc.tile_pool(name="sm_ps", bufs=1, space="PSUM"))
        bc_ps = ctx.enter_context(tc.tile_pool(name="bc_ps", bufs=1, space="PSUM"))

        # ---- resident constants (order = DMA issue order; wkvd first: the
        # first matmuls need it; x^T transposes next; wqd after) ----
        workA = tc.alloc_tile_pool(name="workA", bufs=2)
        xts = {}

        def issue_xt(c):
            xt = workA.tile([128, KH, T], bf16, tag="xt", bufs=2, name=f"xt{c}")
            for i in range(KH):
                nc.sync.dma_start(
                    out=xt[:, i, :],
                    in_=x_ap[ds(i * 128, 128), ds(c * T, T)],
                )
            xts[c] = xt

        issue_xt(0)

        wkvd_sb = const.tile([128, KH, KV_RANK + D_ROPE], bf16, name="wkvd_sb")
        for k in range(KH):
            nc.sync.dma_start(out=wkvd_sb[:, k, :], in_=wkvd_ap[ds(k * 128, 128), :])
        wqd_sb = const.tile([128, KH, Q_RANK], bf16, name="wqd_sb")
        for k in range(KH):
            nc.sync.dma_start(out=wqd_sb[:, k, :], in_=wqd_ap[ds(k * 128, 128), :])
        wqu_sb = const.tile([128, RQ, NH * HEAD_DIM], bf16, name="wqu_sb")
        for r in range(RQ):
            nc.sync.dma_start(out=wqu_sb[:, r, :], in_=wqu_ap[ds(r * 128, 128), :])
        wkvuk_sb = const.tile([128, RKV, NH * D_NOPE], bf16, name="wkvuk_sb")
        for j in range(RKV):
            nc.sync.dma_start(out=wkvuk_sb[:, j, :], in_=wkvuk_ap[ds(j * 128, 128), :])
        wkvuv_sb = const.tile([128, RKV, NH * V_DIM], bf16, name="wkvuv_sb")
        for j in range(RKV):
            nc.sync.dma_start(out=wkvuv_sb[:, j, :], in_=wkvuv_ap[ds(j * 128, 128), :])
        cos_sb = const.tile([128, S], bf16, name="cos_sb")
        nc.sync.dma_start(out=cos_sb[:], in_=cos_ap[:])
        sin_sb = const.tile([128, S], bf16, name="sin_sb")
        nc.sync.dma_start(out=sin_sb[:], in_=sin_ap[:])
        mask_sb = const.tile([128, 1024], bf16, name="mask_sb")
        nc.sync.dma_start(out=mask_sb[:], in_=mask_ap[:])
        onesc_sb = const.tile([128, 1], bf16, name="onesc_sb")
        nc.sync.dma_start(out=onesc_sb[:], in_=onesc_ap[:])
        onesr_f32 = const.tile([1, 128], f32, name="onesr_f32")
        nc.sync.dma_start(out=onesr_f32[:], in_=onesr_ap[:])
        onesr_sb = const.tile([1, 128], dt.float32r, name="onesr_sb")
        with nc.allow_low_precision(reason="exact ones rounded to f32r"):
            nc.vector.tensor_copy(onesr_sb[:], onesr_f32[:])

        # persistent k_rope (rotated, stacked [y1(32); y2(32)])
        krope_sb = const.tile([64, S], bf16, name="krope_sb")
        eps_sb = const.tile([1, 1], f32, name="eps_sb")
        nc.gpsimd.memset(eps_sb[:], EPS)
        # attn^T per head, SBUF-resident through phases B/C
        at_sb = const.tile([128, NH, S], bf16, name="at_sb")

        # DRAM spill buffers
        tk = "ExternalOutput" if taps else "Internal"
        qn_dram = dram.tile([128, NH, S], bf16, name="qn_dram", kind=tk)
        qr_dram = dram.tile([64, NH, S], bf16, name="qr_dram", kind=tk)
        kn_dram = dram.tile([128, NH, S], bf16, name="kn_dram", kind=tk)
        v_dram = dram.tile([128, S // 128, NH * V_DIM], bf16, name="v_dram", kind=tk)
        at_dram = dram.tile([128, NH, S], bf16, name="at_dram", kind=tk) if taps else None

        # ================= phase A: projections =================
        work = workA
        for c in range(NT):
            csl = ds(c * T, T)
            xt = xts.pop(c)
            if c + 1 < NT:
                issue_xt(c + 1)

            # ---- kv down; rmsnorm scalar chain runs under q_down ----
            kvc_bf = work.tile([128, RKV, T], bf16, tag="kvc", bufs=1)
            sq_bf = work.tile([128, RKV, T], bf16, tag="sq", bufs=1)
            for j in range(RKV):
                ps = mm_ps.tile([128, T], f32, tag="mm")
                for k in range(KH):
                    nc.tensor.matmul(
                        ps, wkvd_sb[:, k, ds(j * 128, 128)], xt[:, k, :],
                        start=(k == 0), stop=(k == KH - 1),
                    )
                nc.scalar.activation(sq_bf[:, j, :], ps, AF.Square)
                nc.vector.tensor_copy(kvc_bf[:, j, :], ps)
            ms = sm_ps.tile([1, T], f32, tag="rowps", bufs=2)
            for j in range(RKV):
                nc.tensor.matmul(
                    ms, onesc_sb[:], sq_bf[:, j, :],
                    start=(j == 0), stop=(j == RKV - 1),
                )
            # ---- k rope raw (uses xt; rotation happens later on DVE) ----
            krp = mm_ps.tile([64, T], f32, tag="mm")
            for k in range(KH):
                nc.tensor.matmul(
                    krp, wkvd_sb[:, k, ds(KV_RANK, D_ROPE)], xt[:, k, :],
                    start=(k == 0), stop=(k == KH - 1),
                )
            srt = work.tile([1, T], f32, tag="srt", bufs=1)
            nc.scalar.activation(srt, ms, AF.Sqrt, bias=eps_sb[:], scale=1.0 / KV_RANK)
            rinv = work.tile([1, T], dt.float32r, tag="rinv", bufs=1)
            with nc.allow_low_precision(reason="rsqrt scale rounded to f32r for broadcast matmul"):
                nc.vector.reciprocal(rinv, srt)
            kr_raw = work.tile([64, T], f32, tag="kr_raw", bufs=1)
            nc.vector.tensor_copy(kr_raw, krp)
            kr_sh = work.tile([64, T], f32, tag="kr_sh", bufs=1)
            nc.gpsimd.dma_start(out=kr_sh[0:32, :], in_=kr_raw[32:64, :])
            nc.gpsimd.dma_start(out=kr_sh[32:64, :], in_=kr_raw[0:32, :])
            kt1 = work.tile([64, T], f32, tag="kt1", bufs=1)
            kt2 = work.tile([64, T], f32, tag="kt2", bufs=1)
            nc.vector.tensor_mul(kt1, kr_raw, cos_sb[0:64, csl])
            nc.vector.tensor_mul(kt2, kr_sh, sin_sb[0:64, csl])
            nc.vector.tensor_sub(krope_sb[0:32, csl], kt1[0:32, :], kt2[0:32, :])
            nc.vector.tensor_add(krope_sb[32:64, csl], kt1[32:64, :], kt2[32:64, :])

            # ---- q down: q_lat^T chunk (PE busy while norm chain runs) ----
            qlat = work.tile([128, RQ, T], bf16, tag="qlat", bufs=1)
            for m in range(RQ):
                ps = mm_ps.tile([128, T], f32, tag="mm")
                for k in range(KH):
                    nc.tensor.matmul(
                        ps, wqd_sb[:, k, ds(m * 128, 128)], xt[:, k, :],
                        start=(k == 0), stop=(k == KH - 1),
                    )
                nc.vector.tensor_copy(qlat[:, m, :], ps)

            # ---- kvcn = kvc * rsqrt(ms) (broadcast via K=1 matmul) ----
            rbc_ps = bc_ps.tile([128, T], f32, tag="bc")
            nc.tensor.matmul(rbc_ps, onesr_sb[:], rinv[:], start=True, stop=True)
            rbc = work.tile([128, T], f32, tag="rbc", bufs=1)
            nc.vector.tensor_copy(rbc, rbc_ps)
            kvcn = work.tile([128, RKV, T], bf16, tag="kvcn", bufs=1)
            for j in range(RKV):
                nc.vector.tensor_mul(kvcn[:, j, :], kvc_bf[:, j, :], rbc)

            # ---- q up: nope per head ----
            for m in range(NH):
                ps = mm_ps.tile([128, T], f32, tag="mm")
                for r in range(RQ):
                    nc.tensor.matmul(
                        ps, wqu_sb[:, r, ds(m * 128, 128)], qlat[:, r, :],
                        start=(r == 0), stop=(r == RQ - 1),
                    )
                qnt = work.tile([128, T], bf16, tag="cast", bufs=3)
                nc.vector.tensor_copy(qnt, ps)
                nc.gpsimd.dma_start(out=qn_dram[:, m, csl], in_=qnt)

            # ---- q up: rope (all heads), rotate, scatter per head ----
            ps1 = mm_ps.tile([128, T], f32, tag="mm")
            for r in range(RQ):
                nc.tensor.matmul(
                    ps1, wqu_sb[:, r, ds(NH * D_NOPE, 128)], qlat[:, r, :],
                    start=(r == 0), stop=(r == RQ - 1),
                )
            ps2 = mm_ps.tile([128, T], f32, tag="mm")
            for r in range(RQ):
                nc.tensor.matmul(
                    ps2, wqu_sb[:, r, ds(NH * D_NOPE + 128, 128)], qlat[:, r, :],
                    start=(r == 0), stop=(r == RQ - 1),
                )
            qa = work.tile([128, T], f32, tag="qa", bufs=1)
            qb = work.tile([128, T], f32, tag="qb", bufs=1)
            nc.vector.tensor_mul(qa, ps1, cos_sb[:, csl])
            nc.vector.tensor_mul(qb, ps2, sin_sb[:, csl])
            y1 = work.tile([128, T], bf16, tag="y1", bufs=2)
            nc.vector.tensor_sub(y1, qa, qb)
            qa2 = work.tile([128, T], f32, tag="qa", bufs=1)
            qb2 = work.tile([128, T], f32, tag="qb", bufs=1)
            nc.vector.tensor_mul(qa2, ps2, cos_sb[:, csl])
            nc.vector.tensor_mul(qb2, ps1, sin_sb[:, csl])
            y2 = work.tile([128, T], bf16, tag="y2", bufs=2)
            nc.vector.tensor_add(y2, qa2, qb2)
            for h in range(NH):
                nc.gpsimd.dma_start(out=qr_dram[0:32, h, csl], in_=y1[ds(32 * h, 32), :])
                nc.gpsimd.dma_start(out=qr_dram[32:64, h, csl], in_=y2[ds(32 * h, 32), :])

            # ---- kv up: k_nope^T (feature-major) ----
            for m in range(NH):
                ps = mm_ps.tile([128, T], f32, tag="mm")
                for j in range(RKV):
                    nc.tensor.matmul(
                        ps, wkvuk_sb[:, j, ds(m * 128, 128)], kvcn[:, j, :],
                        start=(j == 0), stop=(j == RKV - 1),
                    )
                knt = work.tile([128, T], bf16, tag="cast", bufs=3)
                nc.vector.tensor_copy(knt, ps)
                nc.gpsimd.dma_start(out=kn_dram[:, m, csl], in_=knt)

            # ---- kv up: v (token-major) ----
            for s2 in range(T // 128):
                ps = mm_ps.tile([128, NH * V_DIM], f32, tag="mm")
                for j in range(RKV):
                    nc.tensor.matmul(
                        ps, kvcn[:, j, ds(s2 * 128, 128)], wkvuv_sb[:, j, :],
                        start=(j == 0), stop=(j == RKV - 1),
                    )
                vt = work.tile([128, NH * V_DIM], bf16, tag="cast", bufs=3)
                nc.vector.tensor_copy(vt, ps)
                nc.gpsimd.dma_start(out=v_dram[:, c * (T // 128) + s2, :], in_=vt)

        # ================= phase B: attention =================
        workA.release()
        workB = tc.alloc_tile_pool(name="workB", bufs=2)
        work = workB

        def drain_norm(st):
            # deferred normalize: by now rec (DVE) has long finished
            h_, qsl_, pv_, rec_ = st
            rb2_ps = bc_ps.tile([128, QC], f32, tag="bc")
            nc.tensor.matmul(rb2_ps, onesr_sb[:], rec_[:], start=True, stop=True)
            rbs = work.tile([128, QC], f32, tag="rbs", bufs=2)
            nc.vector.tensor_copy(rbs, rb2_ps)
            nc.vector.tensor_mul(at_sb[:, h_, qsl_], pv_, rbs)
            if taps:
                nc.sync.dma_start(out=at_dram[:, h_, qsl_], in_=at_sb[:, h_, qsl_])

        norm_pend = []
        for qc in range(NQC):
            qsl = ds(qc * QC, QC)
            nkc = 4 * qc + 4
            for h in range(NH):
                qn_t = work.tile([128, QC], bf16, tag="qn", bufs=2)
                nc.sync.dma_start(out=qn_t, in_=qn_dram[:, h, qsl])
                qr_t = work.tile([64, QC], bf16, tag="qr", bufs=2)
                nc.sync.dma_start(out=qr_t, in_=qr_dram[:, h, qsl])
                pv = pv_ps.tile([128, QC], f32, tag="pv")
                den = sm_ps.tile([1, QC], f32, tag="rowps", bufs=2)
                pend = []
                for kc in range(nkc):
                    kn_t = work.tile([128, 128], bf16, tag="kn", bufs=12)
                    nc.sync.dma_start(out=kn_t, in_=kn_dram[:, h, ds(kc * 128, 128)])
                    v_t = work.tile([128, 128], bf16, tag="vt", bufs=12)
                    nc.sync.dma_start(out=v_t, in_=v_dram[:, kc, ds(h * V_DIM, V_DIM)])
                    sps = mm_ps.tile([128, QC], f32, tag="mm")
                    nc.tensor.matmul(sps, kn_t, qn_t, start=True, stop=False)
                    nc.tensor.matmul(
                        sps, krope_sb[:, ds(kc * 128, 128)], qr_t,
                        start=False, stop=True,
                    )
                    E = work.tile([128, QC], bf16, tag="E", bufs=9)
                    nc.scalar.activation(E, sps, AF.Exp, scale=SCALE)
                    dm = kc - 4 * qc
                    if dm >= 0:
                        nc.vector.tensor_mul(E, E, mask_sb[:, ds(512 - 128 * dm, 512)])
                    pend.append((kc, E, v_t))
                    if len(pend) > 6:  # drain den/pv six kc behind the scores
                        pkc, pE, pvt = pend.pop(0)
                        nc.tensor.matmul(den, onesc_sb[:], pE, start=(pkc == 0), stop=False)
                        nc.tensor.matmul(pv, pvt, pE, start=(pkc == 0), stop=False)
                while pend:
                    last = len(pend) == 1
                    pkc, pE, pvt = pend.pop(0)
                    nc.tensor.matmul(den, onesc_sb[:], pE, start=(pkc == 0), stop=last)
                    nc.tensor.matmul(pv, pvt, pE, start=(pkc == 0), stop=last)
                rec = work.tile([1, QC], dt.float32r, tag="rec", bufs=2)
                with nc.allow_low_precision(reason="softmax denom rounded to f32r for broadcast matmul"):
                    nc.vector.reciprocal(rec, den)
                norm_pend.append((h, qsl, pv, rec))
                if len(norm_pend) > 1:
                    drain_norm(norm_pend.pop(0))
        while norm_pend:
            drain_norm(norm_pend.pop(0))

        # ================= phase C: out-projection =================
        workB.release()
        workC = ctx.enter_context(tc.tile_pool(name="workC", bufs=2))
        work = workC
        wo_ts = []
        for n in range(HID // 512):
            wo_t = work.tile([128, NH, 512], bf16, tag="wo", bufs=4)
            for f in range(NH):
                nc.sync.dma_start(
                    out=wo_t[:, f, :], in_=wout_ap[ds(f * 128, 128), ds(n * 512, 512)]
                )
            wo_ts.append(wo_t)
        for n in range(HID // 512):
            wo_t = wo_ts[n]
            for t16 in range(S // 128):
                ps = mm_ps.tile([128, 512], f32, tag="mm")
                for f in range(NH):
                    nc.tensor.matmul(
                        ps, at_sb[:, f, ds(t16 * 128, 128)], wo_t[:, f, :],
                        start=(f == 0), stop=(f == NH - 1),
                    )
                o_t = work.tile([128, 512], f32, tag="ot", bufs=3)
                nc.vector.tensor_copy(o_t, ps)
                nc.sync.dma_start(
                    out=out_ap[ds(t16 * 128, 128), ds(n * 512, 512)], in_=o_t
                )

    nc.compile()
    return nc


def get_nc():
    if "nc" not in _CACHE:
        _CACHE["nc"] = build_nc()
    return _CACHE["nc"]


def host_inputs(x, w_q_down, w_q_up, w_kv_down, kv_norm_w, w_kv_up, w_out):
    """Build the 8 per-core input shards (host-side prep, numpy only)."""
    bf = ml_dtypes.bfloat16
    x = np.asarray(x, np.float32)
    inv = 1.0 / THETA ** (np.arange(0, D_ROPE, 2, dtype=np.float64) / D_ROPE)
    ang = np.arange(S, dtype=np.float64)[:, None] * inv[None, :]      # (S, 32)
    cosq = np.ascontiguousarray(np.tile(np.cos(ang).T, (4, 1))).astype(bf)  # (128, S)
    sinq = np.ascontiguousarray(np.tile(np.sin(ang).T, (4, 1))).astype(bf)
    maskt = (
        np.arange(1024)[None, :] >= (np.arange(128)[:, None] + 512)
    ).astype(bf)
    ones_col = np.ones((128, 1), bf)
    ones_row = np.ones((1, 128), np.float32)
    wkv_eff = np.asarray(w_kv_up, np.float32) * np.asarray(kv_norm_w, np.float32)[:, None]

    x_bf = [np.ascontiguousarray(x[b].T).astype(bf) for b in range(B)]
    wqd_bf = np.asarray(w_q_down, np.float32).astype(bf)
    wkvd_bf = np.asarray(w_kv_down, np.float32).astype(bf)
    wqu_f = np.asarray(w_q_up, np.float32)
    wout_f = np.asarray(w_out, np.float32)

    in_maps = []
    for ci in range(NCORES):
        b, hg = divmod(ci, 4)
        heads = list(range(NH * hg, NH * hg + NH))
        qu_cols = (
            [h * HEAD_DIM + j for h in heads for j in range(D_NOPE)]
            + [h * HEAD_DIM + D_NOPE + j for h in heads for j in range(32)]
            + [h * HEAD_DIM + D_NOPE + 32 + j for h in heads for j in range(32)]
        )
        kn_cols = [h * (D_NOPE + V_DIM) + j for h in heads for j in range(D_NOPE)]
        v_cols = [h * (D_NOPE + V_DIM) + D_NOPE + j for h in heads for j in range(V_DIM)]
        in_maps.append(
            {
                "x": x_bf[b],
                "wqd": wqd_bf,
                "wqu": np.ascontiguousarray(wqu_f[:, qu_cols]).astype(bf),
                "wkvd": wkvd_bf,
                "wkvuk": np.ascontiguousarray(wkv_eff[:, kn_cols]).astype(bf),
                "wkvuv": np.ascontiguousarray(wkv_eff[:, v_cols]).astype(bf),
                "wout": np.ascontiguousarray(
                    wout_f[NH * V_DIM * hg : NH * V_DIM * (hg + 1), :]
                ).astype(bf),
                "cosq": cosq,
                "sinq": sinq,
                "maskt": maskt,
                "ones_col": ones_col,
                "ones_row": ones_row,
            }
        )
    return in_maps


def run(inputs, trace=False, trace_cores=None):
    from concourse.bass_utils import run_bass_kernel_spmd

    nc = get_nc()
    in_maps = host_inputs(**inputs)
    res = run_bass_kernel_spmd(
        nc,
        in_maps,
        core_ids=list(range(NCORES)),
        trace=trace,
        trace_cores=trace_cores,
    )
    out = np.zeros((B, S, HID), np.float32)
    for ci in range(NCORES):
        out[ci // 4] += res.results[ci]["out"]
    return out, res


def kernel(**inputs):
    out, _ = run(inputs, trace=False)
    return out


# revision 21
# speedup vs baseline: 1.1103x; 1.0464x over previous
"""MLA attention kernel for Trainium2 — 8-core tensor-parallel (self-contained).

Sharding: data-parallel over batch (2) x tensor-parallel over head groups
(4 groups of 4 heads) = 8 cores, SPMD (one NEFF, per-core input shards).
Core ci: batch ci//4, heads [4*(ci%4), 4*(ci%4)+4).

Per-core dataflow (everything feature-major "transposed" so the PE never
needs an on-chip transpose):
  x^T tiles via bf16 DMA-transpose (prefetched one chunk ahead)
  kv^T = wkvd.T @ x^T (rmsnorm sum via ones-matmul, scale broadcast via K=1 matmul)
  q_lat^T = wqd.T @ x^T ; q^T = wqu.T @ q_lat^T ; k_nope^T = wkvuk.T @ kv_c^T
  v (token-major) = kv_c^T.T @ wkvuv
  RoPE on rope rows (DVE); host-permuted weight columns group x1/x2 rows
  scores^T[k,q] = k^T.T @ q^T -> exp (ACT, scale folded) -> causal mask (DVE)
  denom[1,q] = ones.T @ E ; out_h^T[v,q] = v.T @ E   (both pipelined 3 deep)
  normalize via K=1 broadcast matmul of 1/denom, deferred one iteration
  out[t, hid] = attn^T.T @ w_out (token-major, contiguous writes)
Host: sums the 4 partial outputs per batch.
"""

import math

import numpy as np
import ml_dtypes

# ---- problem constants (from the reference model) ----
B, S, HID = 2, 2048, 2048
H, D_NOPE, D_ROPE, V_DIM = 16, 128, 64, 128
KV_RANK, Q_RANK = 512, 1536
HEAD_DIM = D_NOPE + D_ROPE
THETA, EPS = 10000.0, 1e-6
NCORES = 8
NH = 4                    # heads per core
T = 512                   # phase-A token chunk
NT = S // T
QC = 512                  # attention query chunk
NQC = S // QC
KH = HID // 128           # 16 k-chunks over HID
RQ = Q_RANK // 128        # 12 chunks over q rank
RKV = KV_RANK // 128      # 4 chunks over kv rank
SCALE = 1.0 / math.sqrt(HEAD_DIM)

_CACHE = {}


def build_nc(taps=False):
    """Build the Bass/Tile program (one NeuronCore, run SPMD on 8)."""
    from contextlib import ExitStack

    import concourse.mybir as mybir
    import concourse.tile as tile
    from concourse import bacc
    from concourse.bass import ds

    dt = mybir.dt
    AF = mybir.ActivationFunctionType
    bf16 = dt.bfloat16
    f32 = dt.float32

    nc = bacc.Bacc(
        "TRN2",
        target_bir_lowering=False,
        debug=False,
        enable_asserts=False,
        num_devices=NCORES,
    )

    # ---- I/O ----
    x_ap = nc.dram_tensor("x", [HID, S // 4], bf16, kind="ExternalInput").ap()
    wqd_ap = nc.dram_tensor("wqd", [HID, Q_RANK], bf16, kind="ExternalInput").ap()
    wqu_ap = nc.dram_tensor("wqu", [Q_RANK, NH * HEAD_DIM], bf16, kind="ExternalInput").ap()
    wkvd_ap = nc.dram_tensor("wkvd", [HID, KV_RANK + D_ROPE], bf16, kind="ExternalInput").ap()
    wkvuk_ap = nc.dram_tensor("wkvuk", [KV_RANK, NH * D_NOPE], bf16, kind="ExternalInput").ap()
    wkvuv_ap = nc.dram_tensor("wkvuv", [KV_RANK, NH * V_DIM], bf16, kind="ExternalInput").ap()
    wout_ap = nc.dram_tensor("wout", [NH * V_DIM, HID], bf16, kind="ExternalInput").ap()
    cos_ap = nc.dram_tensor("cosq", [128, S], bf16, kind="ExternalInput").ap()
    sin_ap = nc.dram_tensor("sinq", [128, S], bf16, kind="ExternalInput").ap()
    mask_ap = nc.dram_tensor("maskt", [128, 1024], bf16, kind="ExternalInput").ap()
    onesc_ap = nc.dram_tensor("ones_col", [128, 1], bf16, kind="ExternalInput").ap()
    onesr_ap = nc.dram_tensor("ones_row", [1, 128], f32, kind="ExternalInput").ap()
    cosl_ap = nc.dram_tensor("cosl", [128, S // 4], bf16, kind="ExternalInput").ap()
    sinl_ap = nc.dram_tensor("sinl", [128, S // 4], bf16, kind="ExternalInput").ap()
    out_ap = nc.dram_tensor("out", [S, HID], f32, kind="ExternalOutput").ap()

    with tile.TileContext(nc) as tc, ExitStack() as ctx:
        const = ctx.enter_context(tc.tile_pool(name="const", bufs=1))
        dram = ctx.enter_context(tc.tile_pool(name="dram", bufs=1, space="DRAM"))
        mm_ps = ctx.enter_context(tc.tile_pool(name="mm_ps", bufs=3, space="PSUM"))
        pv_ps = ctx.enter_context(tc.tile_pool(name="pv_ps", bufs=2, space="PSUM"))
        sm_ps = ctx.enter_context(tc.tile_pool(name="sm_ps", bufs=1, space="PSUM"))
        bc_ps = ctx.enter_context(tc.tile_pool(name="bc_ps", bufs=1, space="PSUM"))

        # ---- resident constants ----
        TL = S // 4  # local token quarter
        workA = tc.alloc_tile_pool(name="workA", bufs=2)

        # local x^T quarter (host pre-transposed, pre-sharded)
        xt = workA.tile([128, KH, TL], bf16, tag="xt", bufs=1)
        for i in range(KH):
            nc.sync.dma_start(out=xt[:, i, :], in_=x_ap[ds(i * 128, 128), :])

        wkvd_sb = const.tile([128, KH, KV_RANK + D_ROPE], bf16, name="wkvd_sb")
        for k in range(KH):
            nc.sync.dma_start(out=wkvd_sb[:, k, :], in_=wkvd_ap[ds(k * 128, 128), :])
        wqd_sb = const.tile([128, KH, Q_RANK], bf16, name="wqd_sb")
        for k in range(KH):
            nc.sync.dma_start(out=wqd_sb[:, k, :], in_=wqd_ap[ds(k * 128, 128), :])
        wqu_sb = const.tile([128, RQ, NH * HEAD_DIM], bf16, name="wqu_sb")
        for r in range(RQ):
            nc.sync.dma_start(out=wqu_sb[:, r, :], in_=wqu_ap[ds(r * 128, 128), :])
        wkvuk_sb = const.tile([128, RKV, NH * D_NOPE], bf16, name="wkvuk_sb")
        for j in range(RKV):
            nc.sync.dma_start(out=wkvuk_sb[:, j, :], in_=wkvuk_ap[ds(j * 128, 128), :])
        wkvuv_sb = const.tile([128, RKV, NH * V_DIM], bf16, name="wkvuv_sb")
        for j in range(RKV):
            nc.sync.dma_start(out=wkvuv_sb[:, j, :], in_=wkvuv_ap[ds(j * 128, 128), :])
        cos_sb = const.tile([128, S], bf16, name="cos_sb")
        nc.sync.dma_start(out=cos_sb[:], in_=cos_ap[:])
        sin_sb = const.tile([128, S], bf16, name="sin_sb")
        nc.sync.dma_start(out=sin_sb[:], in_=sin_ap[:])
        cosl_sb = const.tile([128, TL], bf16, name="cosl_sb")
        nc.sync.dma_start(out=cosl_sb[:], in_=cosl_ap[:])
        sinl_sb = const.tile([128, TL], bf16, name="sinl_sb")
        nc.sync.dma_start(out=sinl_sb[:], in_=sinl_ap[:])
        mask_sb = const.tile([128, 1024], bf16, name="mask_sb")
        nc.sync.dma_start(out=mask_sb[:], in_=mask_ap[:])
        onesc_sb = const.tile([128, 1], bf16, name="onesc_sb")
        nc.sync.dma_start(out=onesc_sb[:], in_=onesc_ap[:])
        onesr_f32 = const.tile([1, 128], f32, name="onesr_f32")
        nc.sync.dma_start(out=onesr_f32[:], in_=onesr_ap[:])
        onesr_sb = const.tile([1, 128], dt.float32r, name="onesr_sb")
        with nc.allow_low_precision(reason="exact ones rounded to f32r"):
            nc.vector.tensor_copy(onesr_sb[:], onesr_f32[:])

        krope_sb = const.tile([64, S], bf16, name="krope_sb")
        eps_sb = const.tile([1, 1], f32, name="eps_sb")
        nc.gpsimd.memset(eps_sb[:], EPS)
        at_sb = const.tile([128, NH, S], bf16, name="at_sb")

        # DRAM spill + collective bounce buffers
        tk = "ExternalOutput" if taps else "Internal"
        qn_dram = dram.tile([128, NH, S], bf16, name="qn_dram", kind=tk)
        qr_dram = dram.tile([64, NH, S], bf16, name="qr_dram", kind=tk)
        kn_dram = dram.tile([128, NH, S], bf16, name="kn_dram", kind=tk)
        v_dram = dram.tile([128, S // 128, NH * V_DIM], bf16, name="v_dram", kind=tk)
        at_dram = dram.tile([128, NH, S], bf16, name="at_dram", kind=tk) if taps else None
        gin_kv = dram.tile([KV_RANK + D_ROPE, TL], bf16, name="gin_kv")
        gout_kv = dram.tile([4, KV_RANK + D_ROPE, TL], bf16, name="gout_kv")
        gin_q = dram.tile([Q_RANK, TL], bf16, name="gin_q")
        gout_q = dram.tile([4, Q_RANK, TL], bf16, name="gout_q")
        GROUPS = [[0, 1, 2, 3], [4, 5, 6, 7]]

        # ================= phase A0: local down-projections =================
        work = workA
        # ---- kv down (local quarter) ----
        kvc_bf = work.tile([128, RKV, TL], bf16, tag="kvc", bufs=1)
        sq_bf = work.tile([128, RKV, TL], bf16, tag="sq", bufs=1)
        for j in range(RKV):
            ps = mm_ps.tile([128, TL], f32, tag="mm")
            for k in range(KH):
                nc.tensor.matmul(
                    ps, wkvd_sb[:, k, ds(j * 128, 128)], xt[:, k, :],
                    start=(k == 0), stop=(k == KH - 1),
                )
            nc.scalar.activation(sq_bf[:, j, :], ps, AF.Square)
            nc.vector.tensor_copy(kvc_bf[:, j, :], ps)
        ms = sm_ps.tile([1, TL], f32, tag="rowps", bufs=2)
        for j in range(RKV):
            nc.tensor.matmul(
                ms, onesc_sb[:], sq_bf[:, j, :],
                start=(j == 0), stop=(j == RKV - 1),
            )
        krp = mm_ps.tile([64, TL], f32, tag="mm")
        for k in range(KH):
            nc.tensor.matmul(
                krp, wkvd_sb[:, k, ds(KV_RANK, D_ROPE)], xt[:, k, :],
                start=(k == 0), stop=(k == KH - 1),
            )
        srt = work.tile([1, TL], f32, tag="srt", bufs=1)
        nc.scalar.activation(srt, ms, AF.Sqrt, bias=eps_sb[:], scale=1.0 / KV_RANK)
        rinv = work.tile([1, TL], dt.float32r, tag="rinv", bufs=1)
        with nc.allow_low_precision(reason="rsqrt scale rounded to f32r for broadcast matmul"):
            nc.vector.reciprocal(rinv, srt)
        # k rope rotate (local quarter, local cos/sin)
        kr_raw = work.tile([64, TL], f32, tag="kr_raw", bufs=1)
        nc.vector.tensor_copy(kr_raw, krp)
        kr_sh = work.tile([64, TL], f32, tag="kr_sh", bufs=1)
        nc.gpsimd.dma_start(out=kr_sh[0:32, :], in_=kr_raw[32:64, :])
        nc.gpsimd.dma_start(out=kr_sh[32:64, :], in_=kr_raw[0:32, :])
        kt1 = work.tile([64, TL], f32, tag="kt1", bufs=1)
        kt2 = work.tile([64, TL], f32, tag="kt2", bufs=1)
        nc.vector.tensor_mul(kt1, kr_raw, cosl_sb[0:64, :])
        nc.vector.tensor_mul(kt2, kr_sh, sinl_sb[0:64, :])
        krl = work.tile([64, TL], bf16, tag="krl", bufs=1)
        nc.vector.tensor_sub(krl[0:32, :], kt1[0:32, :], kt2[0:32, :])
        nc.vector.tensor_add(krl[32:64, :], kt1[32:64, :], kt2[32:64, :])
        # kvcn = kvc * rsqrt(ms)
        rbc_ps = bc_ps.tile([128, TL], f32, tag="bc")
        nc.tensor.matmul(rbc_ps, onesr_sb[:], rinv[:], start=True, stop=True)
        rbc = work.tile([128, TL], f32, tag="rbc", bufs=1)
        nc.vector.tensor_copy(rbc, rbc_ps)
        kvcn = work.tile([128, RKV, TL], bf16, tag="kvcn", bufs=1)
        for j in range(RKV):
            nc.vector.tensor_mul(kvcn[:, j, :], kvc_bf[:, j, :], rbc)
        # pack + gather kv latents
        for j in range(RKV):
            nc.gpsimd.dma_start(out=gin_kv[ds(j * 128, 128), :], in_=kvcn[:, j, :])
        nc.gpsimd.dma_start(out=gin_kv[ds(KV_RANK, D_ROPE), :], in_=krl[:])
        nc.gpsimd.collective_compute(
            "AllGather", mybir.AluOpType.bypass, replica_groups=GROUPS,
            ins=[gin_kv.opt()], outs=[gout_kv.opt()],
        )

        # ---- q down (local quarter) ----
        qlat = work.tile([128, RQ, TL], bf16, tag="qlat", bufs=1)
        for m in range(RQ):
            ps = mm_ps.tile([128, TL], f32, tag="mm")
            for k in range(KH):
                nc.tensor.matmul(
                    ps, wqd_sb[:, k, ds(m * 128, 128)], xt[:, k, :],
                    start=(k == 0), stop=(k == KH - 1),
                )
            nc.vector.tensor_copy(qlat[:, m, :], ps)
            nc.gpsimd.dma_start(out=gin_q[ds(m * 128, 128), :], in_=qlat[:, m, :])
        nc.gpsimd.collective_compute(
            "AllGather", mybir.AluOpType.bypass, replica_groups=GROUPS,
            ins=[gin_q.opt()], outs=[gout_q.opt()],
        )

        # krope full from gathered blocks
        for c in range(4):
            nc.sync.dma_start(out=krope_sb[:, ds(c * TL, TL)], in_=gout_kv[c, ds(KV_RANK, D_ROPE), :])

        workA.release()
        workA1 = tc.alloc_tile_pool(name="workA1", bufs=2)
        work = workA1

        # ================= phase A1: kv up-projections per chunk =================
        for c in range(NT):
            csl = ds(c * T, T)
            kvg = work.tile([128, RKV, T], bf16, tag="kvg", bufs=2)
            for j in range(RKV):
                nc.sync.dma_start(out=kvg[:, j, :], in_=gout_kv[c, ds(j * 128, 128), :])
            for m in range(NH):
                ps = mm_ps.tile([128, T], f32, tag="mm")
                for j in range(RKV):
                    nc.tensor.matmul(
                        ps, wkvuk_sb[:, j, ds(m * 128, 128)], kvg[:, j, :],
                        start=(j == 0), stop=(j == RKV - 1),
                    )
                knt = work.tile([128, T], bf16, tag="cast", bufs=3)
                nc.vector.tensor_copy(knt, ps)
                nc.gpsimd.dma_start(out=kn_dram[:, m, csl], in_=knt)
            for s2 in range(T // 128):
                ps = mm_ps.tile([128, NH * V_DIM], f32, tag="mm")
                for j in range(RKV):
                    nc.tensor.matmul(
                        ps, kvg[:, j, ds(s2 * 128, 128)], wkvuv_sb[:, j, :],
                        start=(j == 0), stop=(j == RKV - 1),
                    )
                vt = work.tile([128, NH * V_DIM], bf16, tag="cast", bufs=3)
                nc.vector.tensor_copy(vt, ps)
                nc.gpsimd.dma_start(out=v_dram[:, c * (T // 128) + s2, :], in_=vt)

        # ================= phase A2: q up-projections per chunk =================
        for c in range(NT):
            csl = ds(c * T, T)
            qlg = work.tile([128, RQ, T], bf16, tag="qlg", bufs=2)
            for m in range(RQ):
                nc.sync.dma_start(out=qlg[:, m, :], in_=gout_q[c, ds(m * 128, 128), :])
            for m in range(NH):
                ps = mm_ps.tile([128, T], f32, tag="mm")
                for r in range(RQ):
                    nc.tensor.matmul(
                        ps, wqu_sb[:, r, ds(m * 128, 128)], qlg[:, r, :],
                        start=(r == 0), stop=(r == RQ - 1),
                    )
                qnt = work.tile([128, T], bf16, tag="cast", bufs=3)
                nc.vector.tensor_copy(qnt, ps)
                nc.gpsimd.dma_start(out=qn_dram[:, m, csl], in_=qnt)
            ps1 = mm_ps.tile([128, T], f32, tag="mm")
            for r in range(RQ):
                nc.tensor.matmul(
                    ps1, wqu_sb[:, r, ds(NH * D_NOPE, 128)], qlg[:, r, :],
                    start=(r == 0), stop=(r == RQ - 1),
                )
            ps2 = mm_ps.tile([128, T], f32, tag="mm")
            for r in range(RQ):
                nc.tensor.matmul(
                    ps2, wqu_sb[:, r, ds(NH * D_NOPE + 128, 128)], qlg[:, r, :],
                    start=(r == 0), stop=(r == RQ - 1),
                )
            qa = work.tile([128, T], f32, tag="qa", bufs=1)
            qb = work.tile([128, T], f32, tag="qb", bufs=1)
            nc.vector.tensor_mul(qa, ps1, cos_sb[:, csl])
            nc.vector.tensor_mul(qb, ps2, sin_sb[:, csl])
            y1 = work.tile([128, T], bf16, tag="y1", bufs=2)
            nc.vector.tensor_sub(y1, qa, qb)
            qa2 = work.tile([128, T], f32, tag="qa", bufs=1)
            qb2 = work.tile([128, T], f32, tag="qb", bufs=1)
            nc.vector.tensor_mul(qa2, ps2, cos_sb[:, csl])
            nc.vector.tensor_mul(qb2, ps1, sin_sb[:, csl])
            y2 = work.tile([128, T], bf16, tag="y2", bufs=2)
            nc.vector.tensor_add(y2, qa2, qb2)
            for h in range(NH):
                nc.gpsimd.dma_start(out=qr_dram[0:32, h, csl], in_=y1[ds(32 * h, 32), :])
                nc.gpsimd.dma_start(out=qr_dram[32:64, h, csl], in_=y2[ds(32 * h, 32), :])

        # ================= phase B: attention =================
        workA1.release()
        workB = tc.alloc_tile_pool(name="workB", bufs=2)
        work = workB

        def drain_norm(st):
            # deferred normalize: by now rec (DVE) has long finished
            h_, qsl_, pv_, rec_ = st
            rb2_ps = bc_ps.tile([128, QC], f32, tag="bc")
            nc.tensor.matmul(rb2_ps, onesr_sb[:], rec_[:], start=True, stop=True)
            rbs = work.tile([128, QC], f32, tag="rbs", bufs=2)
            nc.vector.tensor_copy(rbs, rb2_ps)
            nc.vector.tensor_mul(at_sb[:, h_, qsl_], pv_, rbs)
            if taps:
                nc.sync.dma_start(out=at_dram[:, h_, qsl_], in_=at_sb[:, h_, qsl_])

        norm_pend = []
        for qc in range(NQC):
            qsl = ds(qc * QC, QC)
            nkc = 4 * qc + 4
            for h in range(NH):
                qn_t = work.tile([128, QC], bf16, tag="qn", bufs=2)
                nc.sync.dma_start(out=qn_t, in_=qn_dram[:, h, qsl])
                qr_t = work.tile([64, QC], bf16, tag="qr", bufs=2)
                nc.sync.dma_start(out=qr_t, in_=qr_dram[:, h, qsl])
                pv = pv_ps.tile([128, QC], f32, tag="pv")
                den = sm_ps.tile([1, QC], f32, tag="rowps", bufs=2)
                pend = []
                for kc in range(nkc):
                    kn_t = work.tile([128, 128], bf16, tag="kn", bufs=12)
                    nc.sync.dma_start(out=kn_t, in_=kn_dram[:, h, ds(kc * 128, 128)])
                    v_t = work.tile([128, 128], bf16, tag="vt", bufs=12)
                    nc.sync.dma_start(out=v_t, in_=v_dram[:, kc, ds(h * V_DIM, V_DIM)])
                    sps = mm_ps.tile([128, QC], f32, tag="mm")
                    nc.tensor.matmul(sps, kn_t, qn_t, start=True, stop=False)
                    nc.tensor.matmul(
                        sps, krope_sb[:, ds(kc * 128, 128)], qr_t,
                        start=False, stop=True,
                    )
                    E = work.tile([128, QC], bf16, tag="E", bufs=9)
                    nc.scalar.activation(E, sps, AF.Exp, scale=SCALE)
                    dm = kc - 4 * qc
                    if dm >= 0:
                        nc.vector.tensor_mul(E, E, mask_sb[:, ds(512 - 128 * dm, 512)])
                    pend.append((kc, E, v_t))
                    if len(pend) > 6:  # drain den/pv six kc behind the scores
                        pkc, pE, pvt = pend.pop(0)
                        nc.tensor.matmul(den, onesc_sb[:], pE, start=(pkc == 0), stop=False)
                        nc.tensor.matmul(pv, pvt, pE, start=(pkc == 0), stop=False)
                while pend:
                    last = len(pend) == 1
                    pkc, pE, pvt = pend.pop(0)
                    nc.tensor.matmul(den, onesc_sb[:], pE, start=(pkc == 0), stop=last)
                    nc.tensor.matmul(pv, pvt, pE, start=(pkc == 0), stop=last)
                rec = work.tile([1, QC], dt.float32r, tag="rec", bufs=2)
                with nc.allow_low_precision(reason="softmax denom rounded to f32r for broadcast matmul"):
                    nc.vector.reciprocal(rec, den)
                norm_pend.append((h, qsl, pv, rec))
                if len(norm_pend) > 1:
                    drain_norm(norm_pend.pop(0))
        while norm_pend:
            drain_norm(norm_pend.pop(0))

        # ================= phase C: out-projection =================
        workB.release()
        workC = ctx.enter_context(tc.tile_pool(name="workC", bufs=2))
        work = workC
        wo_ts = []
        for n in range(HID // 512):
            wo_t = work.tile([128, NH, 512], bf16, tag="wo", bufs=4)
            for f in range(NH):
                nc.sync.dma_start(
                    out=wo_t[:, f, :], in_=wout_ap[ds(f * 128, 128), ds(n * 512, 512)]
                )
            wo_ts.append(wo_t)
        for n in range(HID // 512):
            wo_t = wo_ts[n]
            for t16 in range(S // 128):
                ps = mm_ps.tile([128, 512], f32, tag="mm")
                for f in range(NH):
                    nc.tensor.matmul(
                        ps, at_sb[:, f, ds(t16 * 128, 128)], wo_t[:, f, :],
                        start=(f == 0), stop=(f == NH - 1),
                    )
                o_t = work.tile([128, 512], f32, tag="ot", bufs=3)
                nc.vector.tensor_copy(o_t, ps)
                nc.sync.dma_start(
                    out=out_ap[ds(t16 * 128, 128), ds(n * 512, 512)], in_=o_t
                )

    nc.compile()
    return nc


def get_nc():
    if "nc" not in _CACHE:
        _CACHE["nc"] = build_nc()
    return _CACHE["nc"]


def host_inputs(x, w_q_down, w_q_up, w_kv_down, kv_norm_w, w_kv_up, w_out):
    """Build the 8 per-core input shards (host-side prep, numpy only)."""
    bf = ml_dtypes.bfloat16
    x = np.asarray(x, np.float32)
    inv = 1.0 / THETA ** (np.arange(0, D_ROPE, 2, dtype=np.float64) / D_ROPE)
    ang = np.arange(S, dtype=np.float64)[:, None] * inv[None, :]      # (S, 32)
    cosq = np.ascontiguousarray(np.tile(np.cos(ang).T, (4, 1))).astype(bf)  # (128, S)
    sinq = np.ascontiguousarray(np.tile(np.sin(ang).T, (4, 1))).astype(bf)
    maskt = (
        np.arange(1024)[None, :] >= (np.arange(128)[:, None] + 512)
    ).astype(bf)
    ones_col = np.ones((128, 1), bf)
    ones_row = np.ones((1, 128), np.float32)
    wkv_eff = np.asarray(w_kv_up, np.float32) * np.asarray(kv_norm_w, np.float32)[:, None]

    xT_bf = [np.ascontiguousarray(x[b].T).astype(bf) for b in range(B)]
    wqd_bf = np.asarray(w_q_down, np.float32).astype(bf)
    wkvd_bf = np.asarray(w_kv_down, np.float32).astype(bf)
    wqu_f = np.asarray(w_q_up, np.float32)
    wout_f = np.asarray(w_out, np.float32)

    in_maps = []
    for ci in range(NCORES):
        b, hg = divmod(ci, 4)
        heads = list(range(NH * hg, NH * hg + NH))
        qu_cols = (
            [h * HEAD_DIM + j for h in heads for j in range(D_NOPE)]
            + [h * HEAD_DIM + D_NOPE + j for h in heads for j in range(32)]
            + [h * HEAD_DIM + D_NOPE + 32 + j for h in heads for j in range(32)]
        )
        kn_cols = [h * (D_NOPE + V_DIM) + j for h in heads for j in range(D_NOPE)]
        v_cols = [h * (D_NOPE + V_DIM) + D_NOPE + j for h in heads for j in range(V_DIM)]
        in_maps.append(
            {
                "x": np.ascontiguousarray(xT_bf[b][:, 512 * hg : 512 * (hg + 1)]),
                "cosl": np.ascontiguousarray(cosq[:, 512 * hg : 512 * (hg + 1)]),
                "sinl": np.ascontiguousarray(sinq[:, 512 * hg : 512 * (hg + 1)]),
                "wqd": wqd_bf,
                "wqu": np.ascontiguousarray(wqu_f[:, qu_cols]).astype(bf),
                "wkvd": wkvd_bf,
                "wkvuk": np.ascontiguousarray(wkv_eff[:, kn_cols]).astype(bf),
                "wkvuv": np.ascontiguousarray(wkv_eff[:, v_cols]).astype(bf),
                "wout": np.ascontiguousarray(
                    wout_f[NH * V_DIM * hg : NH * V_DIM * (hg + 1), :]
                ).astype(bf),
                "cosq": cosq,
                "sinq": sinq,
                "maskt": maskt,
                "ones_col": ones_col,
                "ones_row": ones_row,
            }
        )
    return in_maps


def run(inputs, trace=False, trace_cores=None):
    from concourse.bass_utils import run_bass_kernel_spmd

    nc = get_nc()
    in_maps = host_inputs(**inputs)
    res = run_bass_kernel_spmd(
        nc,
        in_maps,
        core_ids=list(range(NCORES)),
        trace=trace,
        trace_cores=trace_cores,
    )
    out = np.zeros((B, S, HID), np.float32)
    for ci in range(NCORES):
        out[ci // 4] += res.results[ci]["out"]
    return out, res


def kernel(**inputs):
    out, _ = run(inputs, trace=False)
    return out
